# revision 19
# baseline (speedup 1.0000x reference)
"""GAT layer (gnn_message_passing) on 8 Trainium2 NeuronCores — V5.

Strategy (dst-partitioned, replicated projection into rotated local tables):
  * Core p owns dst nodes [p*6272, (p+1)*6272) = 49 blocks of 128.
  * Every core computes the full projected table xp = x @ W.T (bf16,
    feature-permuted j = c*4+h, pre-scaled by att_src) plus an 8-col
    attention sidecar (a_src/a_dst per node — att vectors folded into the
    projection weights, so they come straight out of the matmul). The table
    is stored ROTATED per core: local row r holds global node
    (p*6272 + r) mod 50176, so each core's own dst rows are local rows
    0..6271 with core-independent addressing. PSUM->bf16 copies alternate
    between the vector and scalar engines (both idle during phase A).
  * Edges (no self loops) are bucketed per (dst-block, src-parity) and
    gathered per cell via gpsimd dma_gather (512B rows, superrow int16
    indices), rotated across the 4 SWDGE queues so descriptor generation
    runs on all four Q7 cpu pairs concurrently (~4x). Index padding is -1:
    the Q7 ucode trims trailing negatives, so each core gathers only its
    true edge count (cell sizes are padded to the max over cores).
  * One-hot matrices (dst scatter + transpose) are precomputed on the host
    and DMA-streamed per cell, keeping the vector engine free.
  * Per cell: agg += onehot.T @ [w*xp[src] | w] accumulates messages and the
    softmax denominator per block in PSUM; w = exp(leaky(a_src + a_dst)),
    a_src from head-wise row sums of the pre-scaled gathered rows (2x-mode
    tree adds), a_dst via onehot-transpose matmul against the block sidecar.
  * Self loops: the block's own rows + sidecar give msg_self, added at
    finalize. Finalize: normalize, transpose, fused BN+bias (att_src
    unscale folded into BN scale) + ReLU, final linear -> [6272, 64].
"""

import numpy as np
import ml_dtypes

BF16 = ml_dtypes.bfloat16

N, E, F, H, C = 50000, 800000, 256, 4, 64
NEG_SLOPE = 0.2
BN_EPS = 1e-5
NCORES = 8
BLK = 128
NB = 49
OWN = NB * BLK           # 6272
NPAD = NCORES * OWN      # 50176
NT = NPAD // 128         # 392
NTC = NT // 4            # 98 chunks of 4 tiles

# feature permutation: new index j = c*4 + h  <->  old index h*64 + c
_OLD_OF_NEW = (np.arange(F) % H) * C + (np.arange(F) // H)

LAST_EXEC_NS = None
LAST_RESULTS = None


def _prep_edges(edge_index):
    src = np.asarray(edge_index[0], dtype=np.int64)
    dst = np.asarray(edge_index[1], dtype=np.int64)

    core = dst // OWN
    dst_local = dst - core * OWN
    block = dst_local // BLK
    dst_slot = (dst_local % BLK).astype(np.float32)    # identity slot map
    # rotated local row of src on the owning core; parity preserved
    rot = (src[None, :] - (np.arange(NCORES) * OWN)[:, None]) % NPAD  # [8, E]
    cls = (src % 2).astype(np.int64)
    gidx_rot = rot // 2                                # [8, E] superrow/core

    ncell = NB * 2
    cell = core * ncell + block * 2 + cls
    counts = np.bincount(cell, minlength=NCORES * ncell).reshape(NCORES, ncell)
    nie_list = [int(np.ceil(counts[:, ci].max() / 16)) * 16
                for ci in range(ncell)]
    mn_list = [int(counts[:, ci].min()) for ci in range(ncell)]
    subt_list = [(n + 127) // 128 for n in nie_list]
    ni_list = [s * 128 for s in subt_list]
    offs = np.zeros(ncell + 1, dtype=np.int64)
    np.cumsum(ni_list, out=offs[1:])
    TOT = int(offs[-1])

    order = np.argsort(cell, kind="stable")
    sorted_cell = cell[order]
    cell_starts = np.zeros(NCORES * ncell + 1, dtype=np.int64)
    np.cumsum(counts.reshape(-1), out=cell_starts[1:])
    rank = np.arange(len(order)) - cell_starts[sorted_cell]
    ci_of = sorted_cell % ncell
    core_of = sorted_cell // ncell
    flat_pos = offs[ci_of] + rank                      # position within core

    gidx_pad = np.zeros((NCORES, TOT), dtype=np.int64)
    gidx_pad[core_of, flat_pos] = gidx_rot[core_of, order]
    dstm_pad = np.full((NCORES, TOT), 200.0, dtype=np.float32)
    dstm_pad[core_of, flat_pos] = dst_slot[order]

    # wrapped gather indices [16, TOT//16] -> replicated x8 across partitions
    g = gidx_pad.astype(np.int16).reshape(NCORES, TOT // 16, 16)
    g = np.ascontiguousarray(g.transpose(0, 2, 1))
    idx_all = np.tile(g, (1, 8, 1))                    # [8, 128, TOT//16]

    # host-built one-hots, bf16:
    #   oh[p, (t,d)]  = (dstm_gather[p, t] == d)   (partition = edge lane)
    #   ohT[d, (t,e)] = (dstm_flat[t*128+e] == d)  (partition = dst slot)
    dst_w = np.empty((NCORES, 128, TOT // 128), dtype=np.float32)
    for ci in range(ncell):
        seg = dstm_pad[:, offs[ci]:offs[ci + 1]].reshape(
            NCORES, subt_list[ci], 128)
        dst_w[:, :, offs[ci] // 128:offs[ci + 1] // 128] = \
            seg.transpose(0, 2, 1)
    dvals = np.arange(128, dtype=np.float32)
    oh_all = np.empty((NCORES, 128, TOT), dtype=BF16)
    ohT_all = np.empty((NCORES, 128, TOT), dtype=BF16)
    for p in range(NCORES):
        oh = (dst_w[p][:, :, None] == dvals).astype(BF16)   # [128, T/128, 128]
        oh_all[p] = oh.reshape(128, TOT)
        ohT_all[p] = (dvals[:, None] == dstm_pad[p][None, :]).astype(BF16)

    return idx_all, oh_all, ohT_all, (subt_list, nie_list, mn_list)


def _prep_params(x, W, att_src, att_dst, gat_bias, bn_gamma, bn_beta,
                 bn_mean, bn_var, lin_W, lin_b):
    f32 = np.float32
    W = np.asarray(W, f32)
    att_src_f = np.asarray(att_src, f32).reshape(H * C)      # index h*64+c
    att_src_hc = np.asarray(att_src, f32)                    # [H, C]
    att_dst_hc = np.asarray(att_dst, f32)

    wt = W.T                                                 # [in, out_old]
    wt_perm = wt[:, _OLD_OF_NEW] * att_src_f[_OLD_OF_NEW][None, :]
    aw_src = np.zeros((F, H), dtype=f32)
    aw_dst = np.zeros((F, H), dtype=f32)
    for h in range(H):
        aw_src[:, h] = W[h * C:(h + 1) * C, :].T @ att_src_hc[h]
        aw_dst[:, h] = W[h * C:(h + 1) * C, :].T @ att_dst_hc[h]
    wt_full = np.concatenate([wt_perm, aw_src, aw_dst], axis=1)  # [256, 264]
    wt_ext = np.ascontiguousarray(wt_full.reshape(2, 128, 264)).astype(BF16)

    xT = np.zeros((F, NPAD), dtype=f32)
    xT[:, :N] = np.asarray(x, f32).T
    xT_t = np.ascontiguousarray(
        xT.reshape(2, 128, NT, 128).transpose(2, 1, 0, 3)).astype(BF16)

    bnscale = np.asarray(bn_gamma, f32) / np.sqrt(np.asarray(bn_var, f32) + BN_EPS)
    bnshift = ((np.asarray(gat_bias, f32) - np.asarray(bn_mean, f32)) * bnscale
               + np.asarray(bn_beta, f32))
    bnsc_f = bnscale[_OLD_OF_NEW] / att_src_f[_OLD_OF_NEW]   # fold unscale
    bnsc = np.ascontiguousarray(bnsc_f.reshape(2, 128).T)
    bnsh = np.ascontiguousarray(bnshift[_OLD_OF_NEW].reshape(2, 128).T)

    linw = np.asarray(lin_W, f32).T[_OLD_OF_NEW, :]
    linw_t = np.ascontiguousarray(linw.reshape(2, 128, 64)).astype(BF16)
    linb_rep = np.tile(np.asarray(lin_b, f32)[None, :], (128, 1))

    ident_f32 = np.eye(128, dtype=np.float32)

    return dict(xT_t=xT_t, wt_ext=wt_ext, bnsc=bnsc.astype(f32),
                bnsh=bnsh.astype(f32), linw=linw_t, linb=linb_rep.astype(f32),
                ident_f32=ident_f32)


def _build(subt_cfg, queue_map=None):
    import concourse.bacc as bacc
    import concourse.mybir as mybir
    import concourse.tile as tile

    dt = mybir.dt
    subt_list, nie_list, mn_list = subt_cfg
    ni_list = [s * 128 for s in subt_list]
    offs = [0]
    for n in ni_list:
        offs.append(offs[-1] + n)
    TOT = offs[-1]
    SMAX = max(subt_list)

    nc = bacc.Bacc("TRN2", target_bir_lowering=False, debug=False,
                   enable_asserts=False, num_devices=NCORES,
                   num_swdge_queues=4)

    xT4_in = nc.dram_tensor("xT4", [NTC, 128, 4, 2, 128], dt.bfloat16,
                            kind="ExternalInput")
    wt_in = nc.dram_tensor("wt_ext", [2, 128, 264], dt.bfloat16,
                           kind="ExternalInput")
    bnsc_in = nc.dram_tensor("bnsc", [128, 2], dt.float32, kind="ExternalInput")
    bnsh_in = nc.dram_tensor("bnsh", [128, 2], dt.float32, kind="ExternalInput")
    linw_in = nc.dram_tensor("linw", [2, 128, 64], dt.bfloat16, kind="ExternalInput")
    linb_in = nc.dram_tensor("linb", [128, 64], dt.float32, kind="ExternalInput")
    identf_in = nc.dram_tensor("ident_f32", [128, 128], dt.float32, kind="ExternalInput")
    idx_in = nc.dram_tensor("idx", [128, TOT // 16], dt.int16, kind="ExternalInput")
    oh_in = nc.dram_tensor("oh", [128, TOT], dt.bfloat16, kind="ExternalInput")
    ohT_in = nc.dram_tensor("ohT", [128, TOT], dt.bfloat16, kind="ExternalInput")
    out_dram = nc.dram_tensor("out", [OWN, 64], dt.float32, kind="ExternalOutput")

    with tile.TileContext(nc) as tc:
        with (
            tc.tile_pool(name="dram", bufs=1, space="DRAM") as dramp,
            tc.tile_pool(name="const", bufs=1) as constp,
        ):
            xp_tab = dramp.tile([NPAD, 256], dt.bfloat16)
            att_tab = dramp.tile([NPAD, 8], dt.bfloat16)
            sup = xp_tab[:].rearrange("(s two) f -> s (two f)", two=2)
            tabw = xp_tab[:].rearrange("(c j p) f -> c p j f", j=4, p=128)
            attw = att_tab[:].rearrange("(c j p) f -> c p j f", j=4, p=128)

            # ---- consts ----
            wt_sb = constp.tile([128, 2, 264], dt.bfloat16)
            for k in range(2):
                nc.sync.dma_start(out=wt_sb[:, k, :], in_=wt_in[k])
            idx_sb = constp.tile([128, TOT // 16], dt.int16)
            nc.sync.dma_start(out=idx_sb[:], in_=idx_in[:])
            bnsc_sb = constp.tile([128, 2], dt.float32)
            nc.sync.dma_start(out=bnsc_sb[:], in_=bnsc_in[:])
            bnsh_sb = constp.tile([128, 2], dt.float32)
            nc.sync.dma_start(out=bnsh_sb[:], in_=bnsh_in[:])
            linw_sb = constp.tile([128, 2, 64], dt.bfloat16)
            for k in range(2):
                nc.sync.dma_start(out=linw_sb[:, k, :], in_=linw_in[k])
            linb_sb = constp.tile([128, 64], dt.float32)
            nc.sync.dma_start(out=linb_sb[:], in_=linb_in[:])
            identf_sb = constp.tile([128, 128], dt.float32)
            nc.sync.dma_start(out=identf_sb[:], in_=identf_in[:])

            # ---- phase A: replicated projection, 4-tile chunks ----
            with (
                tc.tile_pool(name="proj_sb", bufs=6) as psb,
                tc.tile_pool(name="proj_out", bufs=6) as pxp,
                tc.tile_pool(name="proj_ps", bufs=6, space="PSUM") as pps,
            ):
                for c in range(NTC):
                    xt = psb.tile([128, 4, 2, 128], dt.bfloat16)
                    nc.sync.dma_start(out=xt[:], in_=xT4_in[c])
                    xp4 = pxp.tile([128, 4, 264], dt.bfloat16)
                    for j in range(4):
                        ps = pps.tile([128, 264], dt.float32, space="PSUM")
                        nc.tensor.matmul(out=ps[:], lhsT=xt[:, j, 0, :],
                                         rhs=wt_sb[:, 0, :],
                                         start=True, stop=False)
                        nc.tensor.matmul(out=ps[:], lhsT=xt[:, j, 1, :],
                                         rhs=wt_sb[:, 1, :],
                                         start=False, stop=True)
                        if j % 2 == 0:
                            nc.scalar.activation(
                                xp4[:, j, :], ps[:],
                                mybir.ActivationFunctionType.Copy)
                        else:
                            nc.vector.tensor_copy(out=xp4[:, j, :], in_=ps[:])
                    nc.sync.dma_start(out=tabw[c], in_=xp4[:, :, 0:256])
                    nc.sync.dma_start(out=attw[c], in_=xp4[:, :, 256:264])

            # ---- phase B: per-block pipeline ----
            with (
                tc.tile_pool(name="gsb", bufs=4) as gsb,
                tc.tile_pool(name="ohsb", bufs=6) as ohsb,
                tc.tile_pool(name="msb", bufs=4) as msb,
                tc.tile_pool(name="osb", bufs=3) as osb,
                tc.tile_pool(name="fsb", bufs=2) as fsb,
                tc.tile_pool(name="aggps", bufs=3, space="PSUM") as aggps,
                tc.tile_pool(name="adstps", bufs=2, space="PSUM") as adstps,
                tc.tile_pool(name="tps", bufs=2, space="PSUM") as tps,
                tc.tile_pool(name="finps", bufs=1, space="PSUM") as finps,
            ):
                qctr = [0]
                gather_insts = []

                def gq():
                    i = qctr[0]
                    qctr[0] += 1
                    return queue_map[i] if queue_map is not None else 0

                for b in range(NB):
                    own_x = osb.tile([128, 256], dt.bfloat16, tag="ox")
                    nc.sync.dma_start(
                        out=own_x[:], in_=xp_tab[b * 128:(b + 1) * 128, :])
                    own_a = osb.tile([128, 8], dt.bfloat16, tag="oa")
                    nc.sync.dma_start(
                        out=own_a[:], in_=att_tab[b * 128:(b + 1) * 128, :])

                    agg = aggps.tile([128, 260], dt.float32, space="PSUM")
                    for cls in range(2):
                        ci = b * 2 + cls
                        S = subt_list[ci]
                        NI = ni_list[ci]
                        oE = offs[ci]
                        oW = oE // 16
                        nie = nie_list[ci]
                        xg = gsb.tile([128, SMAX, 256], dt.bfloat16,
                                      tag=f"xg{cls}")
                        src_ap = sup[:, 0:256] if cls == 0 else sup[:, 256:512]
                        if nie < S * 128:
                            nc.vector.memset(xg[:, S - 1, :], 0.0)
                        for g0 in range(0, S, 8):
                            nrem = min(nie - g0 * 128, 1024)
                            if nrem <= 0:
                                break
                            gs = (nrem + 127) // 128
                            gi = nc.gpsimd.dma_gather(
                                out_ap=xg[:, g0:g0 + gs, :], in_ap=src_ap,
                                idxs_ap=idx_sb[:, oW + g0 * 8:
                                               oW + g0 * 8 + (nrem + 15) // 16],
                                num_idxs=nrem, num_idxs_reg=nrem,
                                elem_size=256, elem_step=512, queue_num=gq())
                            gather_insts.append(gi)
                        # host-precomputed one-hots
                        oh = ohsb.tile([128, SMAX, 128], dt.bfloat16, tag="oh")
                        nc.scalar.dma_start(
                            out=oh[:, 0:S, :],
                            in_=oh_in[:, oE:oE + NI].rearrange(
                                "p (t d) -> p t d", d=128))
                        ohT = ohsb.tile([128, SMAX, 128], dt.bfloat16, tag="ohT")
                        nc.scalar.dma_start(
                            out=ohT[:, 0:S, :],
                            in_=ohT_in[:, oE:oE + NI].rearrange(
                                "p (t e) -> p t e", e=128))
                        # a_src: head-wise row sums via 2x-mode tree adds
                        xg4 = xg[:, 0:S, :].rearrange(
                            "p t (c h) -> p t c h", h=H)
                        tr1 = msb.tile([128, SMAX, 32, 4], dt.bfloat16,
                                       tag="tr1")
                        nc.vector.tensor_tensor(
                            out=tr1[:, 0:S, :, :], in0=xg4[:, :, 0:32, :],
                            in1=xg4[:, :, 32:64, :], op=mybir.AluOpType.add)
                        tr2 = msb.tile([128, SMAX, 16, 4], dt.bfloat16,
                                       tag="tr2")
                        nc.vector.tensor_tensor(
                            out=tr2[:, 0:S, :, :], in0=tr1[:, 0:S, 0:16, :],
                            in1=tr1[:, 0:S, 16:32, :], op=mybir.AluOpType.add)
                        tr3 = msb.tile([128, SMAX, 8, 4], dt.bfloat16,
                                       tag="tr3")
                        nc.vector.tensor_tensor(
                            out=tr3[:, 0:S, :, :], in0=tr2[:, 0:S, 0:8, :],
                            in1=tr2[:, 0:S, 8:16, :], op=mybir.AluOpType.add)
                        asrc = msb.tile([128, SMAX, 4], dt.float32, tag="asrc")
                        nc.vector.reduce_sum(
                            out=asrc[:, 0:S, :],
                            in_=tr3[:, 0:S, :, :].rearrange(
                                "p t c h -> p t h c"),
                            axis=mybir.AxisListType.X)
                        # a_dst per edge via one-hot-transpose matmuls
                        adps = adstps.tile([128, SMAX, 4], dt.float32,
                                           space="PSUM")
                        for t in range(S):
                            nc.tensor.matmul(out=adps[:, t, :],
                                             lhsT=ohT[:, t, :],
                                             rhs=own_a[:, 4:8],
                                             start=True, stop=True)
                        # w = exp(leaky(a_src + a_dst)) -> msg cols 256:260
                        ev = msb.tile([128, SMAX, 4], dt.float32, tag="ev")
                        nc.vector.tensor_tensor(out=ev[:, 0:S, :],
                                                in0=asrc[:, 0:S, :],
                                                in1=adps[:, 0:S, :],
                                                op=mybir.AluOpType.add)
                        lv = msb.tile([128, SMAX, 4], dt.float32, tag="lv")
                        nc.vector.scalar_tensor_tensor(
                            out=lv[:, 0:S, :], in0=ev[:, 0:S, :],
                            scalar=NEG_SLOPE, in1=ev[:, 0:S, :],
                            op0=mybir.AluOpType.mult,
                            op1=mybir.AluOpType.max)
                        msg = msb.tile([128, SMAX, 260], dt.bfloat16,
                                       tag="msg")
                        nc.scalar.activation(msg[:, 0:S, 256:260],
                                             lv[:, 0:S, :],
                                             mybir.ActivationFunctionType.Exp)
                        nc.vector.tensor_tensor(
                            out=msg[:, 0:S, 0:256].rearrange(
                                "p t (c h) -> p t c h", h=H),
                            in0=xg4[:],
                            in1=msg[:, 0:S, 256:260][:, :, None, :]
                                .to_broadcast([128, S, C, H]),
                            op=mybir.AluOpType.mult)
                        for t in range(S):
                            nc.tensor.matmul(
                                out=agg[:], lhsT=oh[:, t, :],
                                rhs=msg[:, t, :],
                                start=(cls == 0 and t == 0),
                                stop=(cls == 1 and t == S - 1))
                    # ---- finalize (self loop + normalize + BN + linear) ----
                    evs = fsb.tile([128, 4], dt.float32, tag="evs")
                    nc.vector.tensor_tensor(out=evs[:], in0=own_a[:, 0:4],
                                            in1=own_a[:, 4:8],
                                            op=mybir.AluOpType.add)
                    lvs = fsb.tile([128, 4], dt.float32, tag="lvs")
                    nc.vector.scalar_tensor_tensor(
                        out=lvs[:], in0=evs[:], scalar=NEG_SLOPE, in1=evs[:],
                        op0=mybir.AluOpType.mult, op1=mybir.AluOpType.max)
                    selfmsg = fsb.tile([128, 260], dt.float32, tag="sm")
                    nc.scalar.activation(selfmsg[:, 256:260], lvs[:],
                                         mybir.ActivationFunctionType.Exp)
                    nc.vector.tensor_tensor(
                        out=selfmsg[:, 0:256].rearrange(
                            "p (c h) -> p c h", h=H),
                        in0=own_x[:].rearrange("p (c h) -> p c h", h=H),
                        in1=selfmsg[:, 256:260][:, None, :].to_broadcast(
                            [128, C, H]),
                        op=mybir.AluOpType.mult)
                    tot = fsb.tile([128, 260], dt.float32, tag="tot")
                    nc.vector.tensor_tensor(out=tot[:], in0=agg[:],
                                            in1=selfmsg[:],
                                            op=mybir.AluOpType.add)
                    rec = fsb.tile([128, 4], dt.float32, tag="rec")
                    nc.vector.reciprocal(rec[:], tot[:, 256:260])
                    gat = fsb.tile([128, 256], dt.float32, tag="gat")
                    nc.vector.tensor_tensor(
                        out=gat[:].rearrange("p (c h) -> p c h", h=H),
                        in0=tot[:, 0:256].rearrange("p (c h) -> p c h", h=H),
                        in1=rec[:, None, :].to_broadcast([128, C, H]),
                        op=mybir.AluOpType.mult)
                    fps = finps.tile([128, 64], dt.float32, space="PSUM")
                    gt = fsb.tile([128, 2, 128], dt.bfloat16, tag="gt")
                    for k in range(2):
                        pst = tps.tile([128, 128], dt.float32, space="PSUM",
                                       tag="pst")
                        nc.tensor.transpose(out=pst[:],
                                            in_=gat[:, k * 128:(k + 1) * 128],
                                            identity=identf_sb[:])
                        nc.scalar.activation(gt[:, k, :], pst[:],
                                             mybir.ActivationFunctionType.Relu,
                                             bias=bnsh_sb[:, k:k + 1],
                                             scale=bnsc_sb[:, k:k + 1])
                        nc.tensor.matmul(out=fps[:], lhsT=gt[:, k, :],
                                         rhs=linw_sb[:, k, :],
                                         start=(k == 0), stop=(k == 1))
                    ob = fsb.tile([128, 64], dt.float32, tag="ob")
                    nc.vector.tensor_tensor(out=ob[:], in0=fps[:],
                                            in1=linb_sb[:],
                                            op=mybir.AluOpType.add)
                    nc.sync.dma_start(
                        out=out_dram[b * 128:(b + 1) * 128, :], in_=ob[:])
    nc.compile()
    return nc, gather_insts


def _queue_map_from_lanes(gather_insts):
    """Pass-1 lane readback: queue k must equal (DMASW lane) % 4."""
    from concourse.tile_scheduler import PROC_NAMES
    qmap = []
    for gi in gather_insts:
        ins = getattr(gi, "instruction", gi)
        proc = getattr(ins, "bass_scheduled_proc", None)
        name = PROC_NAMES[proc] if proc is not None else "DMASW0"
        assert name.startswith("DMASW"), name
        qmap.append(int(name[5:]) % 4)
    return qmap


def _install_ntff_shim():
    """Install the axon NTFF profiling hook (missing antenv.axon_hooks shim)."""
    import sys, types
    if "antenv.axon_hooks" in sys.modules:
        return
    m = types.ModuleType("antenv.axon_hooks")
    _h = [None]
    m.set_axon_ntff_profile_hook = lambda h: _h.__setitem__(0, h)
    m.get_axon_ntff_profile_hook = lambda: _h[0]
    sys.modules["antenv.axon_hooks"] = m
    import antenv
    antenv.axon_hooks = m
    from trn_agent_boot.trn_boot import _ntff_profile_via_ctypes
    hook = _ntff_profile_via_ctypes("/opt/axon/libaxon_pjrt.so")
    if hook is not None:
        m.set_axon_ntff_profile_hook(hook)


def kernel(**inputs):
    global LAST_EXEC_NS, LAST_RESULTS
    import os
    from concourse import bass_utils

    trace = os.environ.get("KERNEL_TRACE") == "1"
    if trace:
        try:
            _install_ntff_shim()
            bass_utils.upload_artifacts = lambda tmpdir: "(upload skipped)"
        except Exception as e:
            print("ntff shim failed:", e)
            trace = False

    idx_all, oh_all, ohT_all, subt_cfg = _prep_edges(
        np.asarray(inputs["edge_index"]))
    params = _prep_params(
        inputs["x"], inputs["W"], inputs["att_src"], inputs["att_dst"],
        inputs["gat_bias"], inputs["bn_gamma"], inputs["bn_beta"],
        inputs["bn_mean"], inputs["bn_var"], inputs["lin_W"], inputs["lin_b"])

    nc1, ginsts = _build(subt_cfg)
    nc, _ = _build(subt_cfg, queue_map=_queue_map_from_lanes(ginsts))

    xT_t = params["xT_t"]                    # [NT, 128, 2, 128]
    shared = dict(
        wt_ext=params["wt_ext"], bnsc=params["bnsc"], bnsh=params["bnsh"],
        linw=params["linw"], linb=params["linb"],
        ident_f32=params["ident_f32"])
    in_maps = []
    for p in range(NCORES):
        m = dict(shared)
        rot = np.roll(np.arange(NT), -p * NB)     # tile t holds local rows
        xr = xT_t[rot]
        m["xT4"] = np.ascontiguousarray(
            xr.reshape(NTC, 4, 128, 2, 128).transpose(0, 2, 1, 3, 4))
        m["idx"] = np.ascontiguousarray(idx_all[p])
        m["oh"] = np.ascontiguousarray(oh_all[p])
        m["ohT"] = np.ascontiguousarray(ohT_all[p])
        in_maps.append(m)

    run_kwargs = {}
    if trace:
        run_kwargs = dict(trace=True, tmpdir=os.environ.get(
            "KERNEL_TRACE_DIR", "/tmp/gat_prof"))
        os.makedirs(run_kwargs["tmpdir"], exist_ok=True)
    res = bass_utils.run_bass_kernel_spmd(
        nc, in_maps, core_ids=list(range(NCORES)), **run_kwargs)
    LAST_EXEC_NS = res.exec_time_ns
    LAST_RESULTS = res

    full = np.empty((NPAD, 64), dtype=np.float32)
    for p in range(NCORES):
        full[p * OWN:(p + 1) * OWN] = res.results[p]["out"]
    return full[:N]


# revision 20
# speedup vs baseline: 1.3093x; 1.3093x over previous
"""GAT layer (gnn_message_passing) on 8 Trainium2 NeuronCores — V5.

Strategy (dst-partitioned, replicated projection into rotated local tables):
  * Core p owns dst nodes [p*6272, (p+1)*6272) = 49 blocks of 128.
  * Every core computes the full projected table xp = x @ W.T (bf16,
    feature-permuted j = c*4+h, pre-scaled by att_src) plus an 8-col
    attention sidecar (a_src/a_dst per node — att vectors folded into the
    projection weights, so they come straight out of the matmul). The table
    is stored ROTATED per core: local row r holds global node
    (p*6272 + r) mod 50176, so each core's own dst rows are local rows
    0..6271 with core-independent addressing. PSUM->bf16 copies alternate
    between the vector and scalar engines (both idle during phase A).
  * Edges (no self loops) are bucketed per (dst-block, src-parity) and
    gathered per cell via gpsimd dma_gather (512B rows, superrow int16
    indices), rotated across the 4 SWDGE queues so descriptor generation
    runs on all four Q7 cpu pairs concurrently (~4x). Index padding is -1:
    the Q7 ucode trims trailing negatives, so each core gathers only its
    true edge count (cell sizes are padded to the max over cores).
  * One-hot matrices (dst scatter + transpose) are precomputed on the host
    and DMA-streamed per cell, keeping the vector engine free.
  * Per cell: agg += onehot.T @ [w*xp[src] | w] accumulates messages and the
    softmax denominator per block in PSUM; w = exp(leaky(a_src + a_dst)),
    a_src from head-wise row sums of the pre-scaled gathered rows (2x-mode
    tree adds), a_dst via onehot-transpose matmul against the block sidecar.
  * Self loops: the block's own rows + sidecar give msg_self, added at
    finalize. Finalize: normalize, transpose, fused BN+bias (att_src
    unscale folded into BN scale) + ReLU, final linear -> [6272, 64].
"""

import numpy as np
import ml_dtypes

BF16 = ml_dtypes.bfloat16

N, E, F, H, C = 50000, 800000, 256, 4, 64
NEG_SLOPE = 0.2
BN_EPS = 1e-5
NCORES = 8
BLK = 128
NB = 49
OWN = NB * BLK           # 6272
NPAD = NCORES * OWN      # 50176
NT = NPAD // 128         # 392
NTC = NT // 4            # 98 chunks of 4 tiles

# feature permutation: new index j = c*4 + h  <->  old index h*64 + c
_OLD_OF_NEW = (np.arange(F) % H) * C + (np.arange(F) // H)

LAST_EXEC_NS = None
LAST_RESULTS = None


def _prep_edges(edge_index):
    src = np.asarray(edge_index[0], dtype=np.int64)
    dst = np.asarray(edge_index[1], dtype=np.int64)

    core = dst // OWN
    dst_local = dst - core * OWN
    block = dst_local // BLK
    dst_slot = (dst_local % BLK).astype(np.float32)    # identity slot map
    # rotated local row of src on the owning core; parity preserved
    rot = (src[None, :] - (np.arange(NCORES) * OWN)[:, None]) % NPAD  # [8, E]
    cls = (src % 2).astype(np.int64)
    gidx_rot = rot // 2                                # [8, E] superrow/core

    ncell = NB * 2
    cell = core * ncell + block * 2 + cls
    counts = np.bincount(cell, minlength=NCORES * ncell).reshape(NCORES, ncell)
    nie_list = [int(np.ceil(counts[:, ci].max() / 16)) * 16
                for ci in range(ncell)]
    mn_list = [int(counts[:, ci].min()) for ci in range(ncell)]
    subt_list = [(n + 127) // 128 for n in nie_list]
    ni_list = [s * 128 for s in subt_list]
    offs = np.zeros(ncell + 1, dtype=np.int64)
    np.cumsum(ni_list, out=offs[1:])
    TOT = int(offs[-1])

    order = np.argsort(cell, kind="stable")
    sorted_cell = cell[order]
    cell_starts = np.zeros(NCORES * ncell + 1, dtype=np.int64)
    np.cumsum(counts.reshape(-1), out=cell_starts[1:])
    rank = np.arange(len(order)) - cell_starts[sorted_cell]
    ci_of = sorted_cell % ncell
    core_of = sorted_cell // ncell
    flat_pos = offs[ci_of] + rank                      # position within core

    gidx_pad = np.zeros((NCORES, TOT), dtype=np.int64)
    gidx_pad[core_of, flat_pos] = gidx_rot[core_of, order]
    dstm_pad = np.full((NCORES, TOT), 200.0, dtype=np.float32)
    dstm_pad[core_of, flat_pos] = dst_slot[order]

    # wrapped gather indices [16, TOT//16] -> replicated x8 across partitions
    g = gidx_pad.astype(np.int16).reshape(NCORES, TOT // 16, 16)
    g = np.ascontiguousarray(g.transpose(0, 2, 1))
    idx_all = np.tile(g, (1, 8, 1))                    # [8, 128, TOT//16]

    # host-built one-hots, bf16:
    #   oh[p, (t,d)]  = (dstm_gather[p, t] == d)   (partition = edge lane)
    #   ohT[d, (t,e)] = (dstm_flat[t*128+e] == d)  (partition = dst slot)
    dst_w = np.empty((NCORES, 128, TOT // 128), dtype=np.float32)
    for ci in range(ncell):
        seg = dstm_pad[:, offs[ci]:offs[ci + 1]].reshape(
            NCORES, subt_list[ci], 128)
        dst_w[:, :, offs[ci] // 128:offs[ci + 1] // 128] = \
            seg.transpose(0, 2, 1)
    dvals = np.arange(128, dtype=np.float32)
    oh_all = np.empty((NCORES, 128, TOT), dtype=BF16)
    ohT_all = np.empty((NCORES, 128, TOT), dtype=BF16)
    for p in range(NCORES):
        oh = (dst_w[p][:, :, None] == dvals).astype(BF16)   # [128, T/128, 128]
        oh_all[p] = oh.reshape(128, TOT)
        ohT_all[p] = (dvals[:, None] == dstm_pad[p][None, :]).astype(BF16)

    return idx_all, oh_all, ohT_all, (subt_list, nie_list, mn_list)


def _prep_params(x, W, att_src, att_dst, gat_bias, bn_gamma, bn_beta,
                 bn_mean, bn_var, lin_W, lin_b):
    f32 = np.float32
    W = np.asarray(W, f32)
    att_src_f = np.asarray(att_src, f32).reshape(H * C)      # index h*64+c
    att_src_hc = np.asarray(att_src, f32)                    # [H, C]
    att_dst_hc = np.asarray(att_dst, f32)

    wt = W.T                                                 # [in, out_old]
    wt_perm = wt[:, _OLD_OF_NEW] * att_src_f[_OLD_OF_NEW][None, :]
    aw_src = np.zeros((F, H), dtype=f32)
    aw_dst = np.zeros((F, H), dtype=f32)
    for h in range(H):
        aw_src[:, h] = W[h * C:(h + 1) * C, :].T @ att_src_hc[h]
        aw_dst[:, h] = W[h * C:(h + 1) * C, :].T @ att_dst_hc[h]
    wt_full = np.concatenate([wt_perm, aw_src, aw_dst], axis=1)  # [256, 264]
    wt_ext = np.ascontiguousarray(wt_full.reshape(2, 128, 264)).astype(BF16)

    xT = np.zeros((F, NPAD), dtype=f32)
    xT[:, :N] = np.asarray(x, f32).T
    xT_t = np.ascontiguousarray(
        xT.reshape(2, 128, NT, 128).transpose(2, 1, 0, 3)).astype(BF16)

    bnscale = np.asarray(bn_gamma, f32) / np.sqrt(np.asarray(bn_var, f32) + BN_EPS)
    bnshift = ((np.asarray(gat_bias, f32) - np.asarray(bn_mean, f32)) * bnscale
               + np.asarray(bn_beta, f32))
    bnsc_f = bnscale[_OLD_OF_NEW] / att_src_f[_OLD_OF_NEW]   # fold unscale
    bnsc = np.ascontiguousarray(bnsc_f.reshape(2, 128).T)
    bnsh = np.ascontiguousarray(bnshift[_OLD_OF_NEW].reshape(2, 128).T)

    linw = np.asarray(lin_W, f32).T[_OLD_OF_NEW, :]
    linw_t = np.ascontiguousarray(linw.reshape(2, 128, 64)).astype(BF16)
    linb_rep = np.tile(np.asarray(lin_b, f32)[None, :], (128, 1))

    ident_f32 = np.eye(128, dtype=np.float32)

    return dict(xT_t=xT_t, wt_ext=wt_ext, bnsc=bnsc.astype(f32),
                bnsh=bnsh.astype(f32), linw=linw_t, linb=linb_rep.astype(f32),
                ident_f32=ident_f32)


def _build(subt_cfg, queue_map=None):
    import concourse.bacc as bacc
    import concourse.mybir as mybir
    import concourse.tile as tile

    dt = mybir.dt
    subt_list, nie_list, mn_list = subt_cfg
    ni_list = [s * 128 for s in subt_list]
    offs = [0]
    for n in ni_list:
        offs.append(offs[-1] + n)
    TOT = offs[-1]
    SMAX = max(subt_list)

    nc = bacc.Bacc("TRN2", target_bir_lowering=False, debug=False,
                   enable_asserts=False, num_devices=NCORES,
                   num_swdge_queues=4)

    xT4_in = nc.dram_tensor("xT4", [NTC, 128, 4, 2, 128], dt.bfloat16,
                            kind="ExternalInput")
    wt_in = nc.dram_tensor("wt_ext", [2, 128, 264], dt.bfloat16,
                           kind="ExternalInput")
    bnsc_in = nc.dram_tensor("bnsc", [128, 2], dt.float32, kind="ExternalInput")
    bnsh_in = nc.dram_tensor("bnsh", [128, 2], dt.float32, kind="ExternalInput")
    linw_in = nc.dram_tensor("linw", [2, 128, 64], dt.bfloat16, kind="ExternalInput")
    linb_in = nc.dram_tensor("linb", [128, 64], dt.float32, kind="ExternalInput")
    identf_in = nc.dram_tensor("ident_f32", [128, 128], dt.float32, kind="ExternalInput")
    idx_in = nc.dram_tensor("idx", [128, TOT // 16], dt.int16, kind="ExternalInput")
    oh_in = nc.dram_tensor("oh", [128, TOT], dt.bfloat16, kind="ExternalInput")
    ohT_in = nc.dram_tensor("ohT", [128, TOT], dt.bfloat16, kind="ExternalInput")
    out_dram = nc.dram_tensor("out", [OWN, 64], dt.float32, kind="ExternalOutput")

    with tile.TileContext(nc) as tc:
        with (
            tc.tile_pool(name="dram", bufs=1, space="DRAM") as dramp,
            tc.tile_pool(name="const", bufs=1) as constp,
        ):
            xp_tab = dramp.tile([NPAD, 256], dt.bfloat16)
            att_tab = dramp.tile([NPAD, 8], dt.bfloat16)
            sup = xp_tab[:].rearrange("(s two) f -> s (two f)", two=2)
            tabw = xp_tab[:].rearrange("(c j p) f -> c p j f", j=4, p=128)
            attw = att_tab[:].rearrange("(c j p) f -> c p j f", j=4, p=128)

            # ---- consts ----
            wt_sb = constp.tile([128, 2, 264], dt.bfloat16)
            for k in range(2):
                nc.sync.dma_start(out=wt_sb[:, k, :], in_=wt_in[k])
            idx_sb = constp.tile([128, TOT // 16], dt.int16)
            nc.sync.dma_start(out=idx_sb[:], in_=idx_in[:])
            bnsc_sb = constp.tile([128, 2], dt.float32)
            nc.sync.dma_start(out=bnsc_sb[:], in_=bnsc_in[:])
            bnsh_sb = constp.tile([128, 2], dt.float32)
            nc.sync.dma_start(out=bnsh_sb[:], in_=bnsh_in[:])
            linw_sb = constp.tile([128, 2, 64], dt.bfloat16)
            for k in range(2):
                nc.sync.dma_start(out=linw_sb[:, k, :], in_=linw_in[k])
            linb_sb = constp.tile([128, 64], dt.float32)
            nc.sync.dma_start(out=linb_sb[:], in_=linb_in[:])
            identf_sb = constp.tile([128, 128], dt.float32)
            nc.sync.dma_start(out=identf_sb[:], in_=identf_in[:])

            # ---- phase A: replicated projection, 4-tile chunks ----
            with (
                tc.tile_pool(name="proj_sb", bufs=6) as psb,
                tc.tile_pool(name="proj_out", bufs=6) as pxp,
                tc.tile_pool(name="proj_ps", bufs=6, space="PSUM") as pps,
            ):
                for c in range(NTC):
                    xt = psb.tile([128, 4, 2, 128], dt.bfloat16)
                    nc.sync.dma_start(out=xt[:], in_=xT4_in[c])
                    xp4 = pxp.tile([128, 4, 264], dt.bfloat16)
                    for j in range(4):
                        ps = pps.tile([128, 264], dt.float32, space="PSUM")
                        nc.tensor.matmul(out=ps[:], lhsT=xt[:, j, 0, :],
                                         rhs=wt_sb[:, 0, :],
                                         start=True, stop=False)
                        nc.tensor.matmul(out=ps[:], lhsT=xt[:, j, 1, :],
                                         rhs=wt_sb[:, 1, :],
                                         start=False, stop=True)
                        if j % 2 == 0:
                            nc.scalar.activation(
                                xp4[:, j, :], ps[:],
                                mybir.ActivationFunctionType.Copy)
                        else:
                            nc.vector.tensor_copy(out=xp4[:, j, :], in_=ps[:])
                    nc.sync.dma_start(out=tabw[c], in_=xp4[:, :, 0:256])
                    nc.sync.dma_start(out=attw[c], in_=xp4[:, :, 256:264])

            # ---- phase B: per-block pipeline ----
            with (
                tc.tile_pool(name="gsb", bufs=4) as gsb,
                tc.tile_pool(name="ohsb", bufs=6) as ohsb,
                tc.tile_pool(name="msb", bufs=4) as msb,
                tc.tile_pool(name="osb", bufs=3) as osb,
                tc.tile_pool(name="fsb", bufs=2) as fsb,
                tc.tile_pool(name="aggps", bufs=3, space="PSUM") as aggps,
                tc.tile_pool(name="adstps", bufs=2, space="PSUM") as adstps,
                tc.tile_pool(name="tps", bufs=2, space="PSUM") as tps,
                tc.tile_pool(name="finps", bufs=1, space="PSUM") as finps,
            ):
                qctr = [0]
                gather_insts = []

                def gq():
                    i = qctr[0]
                    qctr[0] += 1
                    return queue_map[i] if queue_map is not None else 0

                for b in range(NB):
                    own_x = osb.tile([128, 256], dt.bfloat16, tag="ox")
                    nc.sync.dma_start(
                        out=own_x[:], in_=xp_tab[b * 128:(b + 1) * 128, :])
                    own_a = osb.tile([128, 8], dt.bfloat16, tag="oa")
                    nc.sync.dma_start(
                        out=own_a[:], in_=att_tab[b * 128:(b + 1) * 128, :])

                    agg = aggps.tile([128, 260], dt.float32, space="PSUM")
                    for cls in range(2):
                        ci = b * 2 + cls
                        S = subt_list[ci]
                        NI = ni_list[ci]
                        oE = offs[ci]
                        oW = oE // 16
                        nie = nie_list[ci]
                        xg = gsb.tile([128, SMAX, 256], dt.bfloat16,
                                      tag=f"xg{cls}")
                        src_ap = sup[:, 0:256] if cls == 0 else sup[:, 256:512]
                        if nie < S * 128:
                            nc.vector.memset(xg[:, S - 1, :], 0.0)
                        for g0 in range(0, S, 8):
                            nrem = min(nie - g0 * 128, 1024)
                            if nrem <= 0:
                                break
                            gs = (nrem + 127) // 128
                            gi = nc.gpsimd.dma_gather(
                                out_ap=xg[:, g0:g0 + gs, :], in_ap=src_ap,
                                idxs_ap=idx_sb[:, oW + g0 * 8:
                                               oW + g0 * 8 + (nrem + 15) // 16],
                                num_idxs=nrem, num_idxs_reg=nrem,
                                elem_size=256, elem_step=512, queue_num=gq())
                            gather_insts.append(gi)
                        # host-precomputed one-hots
                        oh = ohsb.tile([128, SMAX, 128], dt.bfloat16, tag="oh")
                        nc.scalar.dma_start(
                            out=oh[:, 0:S, :],
                            in_=oh_in[:, oE:oE + NI].rearrange(
                                "p (t d) -> p t d", d=128))
                        ohT = ohsb.tile([128, SMAX, 128], dt.bfloat16, tag="ohT")
                        nc.scalar.dma_start(
                            out=ohT[:, 0:S, :],
                            in_=ohT_in[:, oE:oE + NI].rearrange(
                                "p (t e) -> p t e", e=128))
                        # a_src: head-wise row sums via 2x-mode tree adds
                        xg4 = xg[:, 0:S, :].rearrange(
                            "p t (c h) -> p t c h", h=H)
                        tr1 = msb.tile([128, SMAX, 32, 4], dt.bfloat16,
                                       tag="tr1")
                        nc.vector.tensor_tensor(
                            out=tr1[:, 0:S, :, :], in0=xg4[:, :, 0:32, :],
                            in1=xg4[:, :, 32:64, :], op=mybir.AluOpType.add)
                        tr2 = msb.tile([128, SMAX, 16, 4], dt.bfloat16,
                                       tag="tr2")
                        nc.vector.tensor_tensor(
                            out=tr2[:, 0:S, :, :], in0=tr1[:, 0:S, 0:16, :],
                            in1=tr1[:, 0:S, 16:32, :], op=mybir.AluOpType.add)
                        tr3 = msb.tile([128, SMAX, 8, 4], dt.bfloat16,
                                       tag="tr3")
                        nc.vector.tensor_tensor(
                            out=tr3[:, 0:S, :, :], in0=tr2[:, 0:S, 0:8, :],
                            in1=tr2[:, 0:S, 8:16, :], op=mybir.AluOpType.add)
                        asrc = msb.tile([128, SMAX, 4], dt.float32, tag="asrc")
                        nc.vector.reduce_sum(
                            out=asrc[:, 0:S, :],
                            in_=tr3[:, 0:S, :, :].rearrange(
                                "p t c h -> p t h c"),
                            axis=mybir.AxisListType.X)
                        # a_dst per edge via one-hot-transpose matmuls
                        adps = adstps.tile([128, SMAX, 4], dt.float32,
                                           space="PSUM")
                        for t in range(S):
                            nc.tensor.matmul(out=adps[:, t, :],
                                             lhsT=ohT[:, t, :],
                                             rhs=own_a[:, 4:8],
                                             start=True, stop=True)
                        # w = exp(leaky(a_src + a_dst)) -> msg cols 256:260
                        ev = msb.tile([128, SMAX, 4], dt.float32, tag="ev")
                        nc.vector.tensor_tensor(out=ev[:, 0:S, :],
                                                in0=asrc[:, 0:S, :],
                                                in1=adps[:, 0:S, :],
                                                op=mybir.AluOpType.add)
                        lv = msb.tile([128, SMAX, 4], dt.float32, tag="lv")
                        nc.vector.scalar_tensor_tensor(
                            out=lv[:, 0:S, :], in0=ev[:, 0:S, :],
                            scalar=NEG_SLOPE, in1=ev[:, 0:S, :],
                            op0=mybir.AluOpType.mult,
                            op1=mybir.AluOpType.max)
                        msg = msb.tile([128, SMAX, 260], dt.bfloat16,
                                       tag="msg")
                        nc.scalar.activation(msg[:, 0:S, 256:260],
                                             lv[:, 0:S, :],
                                             mybir.ActivationFunctionType.Exp)
                        nc.vector.tensor_tensor(
                            out=msg[:, 0:S, 0:256].rearrange(
                                "p t (c h) -> p t c h", h=H),
                            in0=xg4[:],
                            in1=msg[:, 0:S, 256:260][:, :, None, :]
                                .to_broadcast([128, S, C, H]),
                            op=mybir.AluOpType.mult)
                        for t in range(S):
                            nc.tensor.matmul(
                                out=agg[:], lhsT=oh[:, t, :],
                                rhs=msg[:, t, :],
                                start=(cls == 0 and t == 0),
                                stop=(cls == 1 and t == S - 1))
                    # ---- finalize (self loop + normalize + BN + linear) ----
                    evs = fsb.tile([128, 4], dt.float32, tag="evs")
                    nc.vector.tensor_tensor(out=evs[:], in0=own_a[:, 0:4],
                                            in1=own_a[:, 4:8],
                                            op=mybir.AluOpType.add)
                    lvs = fsb.tile([128, 4], dt.float32, tag="lvs")
                    nc.vector.scalar_tensor_tensor(
                        out=lvs[:], in0=evs[:], scalar=NEG_SLOPE, in1=evs[:],
                        op0=mybir.AluOpType.mult, op1=mybir.AluOpType.max)
                    selfmsg = fsb.tile([128, 260], dt.float32, tag="sm")
                    nc.scalar.activation(selfmsg[:, 256:260], lvs[:],
                                         mybir.ActivationFunctionType.Exp)
                    nc.vector.tensor_tensor(
                        out=selfmsg[:, 0:256].rearrange(
                            "p (c h) -> p c h", h=H),
                        in0=own_x[:].rearrange("p (c h) -> p c h", h=H),
                        in1=selfmsg[:, 256:260][:, None, :].to_broadcast(
                            [128, C, H]),
                        op=mybir.AluOpType.mult)
                    tot = fsb.tile([128, 260], dt.float32, tag="tot")
                    nc.vector.tensor_tensor(out=tot[:], in0=agg[:],
                                            in1=selfmsg[:],
                                            op=mybir.AluOpType.add)
                    rec = fsb.tile([128, 4], dt.float32, tag="rec")
                    nc.vector.reciprocal(rec[:], tot[:, 256:260])
                    gat = fsb.tile([128, 256], dt.float32, tag="gat")
                    nc.vector.tensor_tensor(
                        out=gat[:].rearrange("p (c h) -> p c h", h=H),
                        in0=tot[:, 0:256].rearrange("p (c h) -> p c h", h=H),
                        in1=rec[:, None, :].to_broadcast([128, C, H]),
                        op=mybir.AluOpType.mult)
                    fps = finps.tile([128, 64], dt.float32, space="PSUM")
                    gt = fsb.tile([128, 2, 128], dt.bfloat16, tag="gt")
                    for k in range(2):
                        pst = tps.tile([128, 128], dt.float32, space="PSUM",
                                       tag="pst")
                        nc.tensor.transpose(out=pst[:],
                                            in_=gat[:, k * 128:(k + 1) * 128],
                                            identity=identf_sb[:])
                        nc.scalar.activation(gt[:, k, :], pst[:],
                                             mybir.ActivationFunctionType.Relu,
                                             bias=bnsh_sb[:, k:k + 1],
                                             scale=bnsc_sb[:, k:k + 1])
                        nc.tensor.matmul(out=fps[:], lhsT=gt[:, k, :],
                                         rhs=linw_sb[:, k, :],
                                         start=(k == 0), stop=(k == 1))
                    ob = fsb.tile([128, 64], dt.float32, tag="ob")
                    nc.vector.tensor_tensor(out=ob[:], in0=fps[:],
                                            in1=linb_sb[:],
                                            op=mybir.AluOpType.add)
                    nc.sync.dma_start(
                        out=out_dram[b * 128:(b + 1) * 128, :], in_=ob[:])
    nc.compile()
    return nc, gather_insts


def _queue_map_from_lanes(gather_insts):
    """Pass-1 lane readback: queue k must equal (DMASW lane) % 4."""
    from concourse.tile_scheduler import PROC_NAMES
    qmap = []
    for gi in gather_insts:
        name = PROC_NAMES[gi.ins.bass_scheduled_proc]
        assert name.startswith("DMASW"), name
        qmap.append(int(name[5:]) % 4)
    return qmap


def _install_ntff_shim():
    """Install the axon NTFF profiling hook (missing antenv.axon_hooks shim)."""
    import sys, types
    if "antenv.axon_hooks" in sys.modules:
        return
    m = types.ModuleType("antenv.axon_hooks")
    _h = [None]
    m.set_axon_ntff_profile_hook = lambda h: _h.__setitem__(0, h)
    m.get_axon_ntff_profile_hook = lambda: _h[0]
    sys.modules["antenv.axon_hooks"] = m
    import antenv
    antenv.axon_hooks = m
    from trn_agent_boot.trn_boot import _ntff_profile_via_ctypes
    hook = _ntff_profile_via_ctypes("/opt/axon/libaxon_pjrt.so")
    if hook is not None:
        m.set_axon_ntff_profile_hook(hook)


def kernel(**inputs):
    global LAST_EXEC_NS, LAST_RESULTS
    import os
    from concourse import bass_utils

    trace = os.environ.get("KERNEL_TRACE") == "1"
    if trace:
        try:
            _install_ntff_shim()
            bass_utils.upload_artifacts = lambda tmpdir: "(upload skipped)"
        except Exception as e:
            print("ntff shim failed:", e)
            trace = False

    idx_all, oh_all, ohT_all, subt_cfg = _prep_edges(
        np.asarray(inputs["edge_index"]))
    params = _prep_params(
        inputs["x"], inputs["W"], inputs["att_src"], inputs["att_dst"],
        inputs["gat_bias"], inputs["bn_gamma"], inputs["bn_beta"],
        inputs["bn_mean"], inputs["bn_var"], inputs["lin_W"], inputs["lin_b"])

    nc1, ginsts = _build(subt_cfg)
    nc, _ = _build(subt_cfg, queue_map=_queue_map_from_lanes(ginsts))

    xT_t = params["xT_t"]                    # [NT, 128, 2, 128]
    shared = dict(
        wt_ext=params["wt_ext"], bnsc=params["bnsc"], bnsh=params["bnsh"],
        linw=params["linw"], linb=params["linb"],
        ident_f32=params["ident_f32"])
    in_maps = []
    for p in range(NCORES):
        m = dict(shared)
        rot = np.roll(np.arange(NT), -p * NB)     # tile t holds local rows
        xr = xT_t[rot]
        m["xT4"] = np.ascontiguousarray(
            xr.reshape(NTC, 4, 128, 2, 128).transpose(0, 2, 1, 3, 4))
        m["idx"] = np.ascontiguousarray(idx_all[p])
        m["oh"] = np.ascontiguousarray(oh_all[p])
        m["ohT"] = np.ascontiguousarray(ohT_all[p])
        in_maps.append(m)

    run_kwargs = {}
    if trace:
        run_kwargs = dict(trace=True, tmpdir=os.environ.get(
            "KERNEL_TRACE_DIR", "/tmp/gat_prof"))
        os.makedirs(run_kwargs["tmpdir"], exist_ok=True)
    res = bass_utils.run_bass_kernel_spmd(
        nc, in_maps, core_ids=list(range(NCORES)), **run_kwargs)
    LAST_EXEC_NS = res.exec_time_ns
    LAST_RESULTS = res

    full = np.empty((NPAD, 64), dtype=np.float32)
    for p in range(NCORES):
        full[p * OWN:(p + 1) * OWN] = res.results[p]["out"]
    return full[:N]


# revision 21
# speedup vs baseline: 1.5854x; 1.2109x over previous
"""GAT layer (gnn_message_passing) on 8 Trainium2 NeuronCores — V5.

Strategy (dst-partitioned, replicated projection into rotated local tables):
  * Core p owns dst nodes [p*6272, (p+1)*6272) = 49 blocks of 128.
  * Every core computes the full projected table xp = x @ W.T (bf16,
    feature-permuted j = c*4+h, pre-scaled by att_src) plus an 8-col
    attention sidecar (a_src/a_dst per node — att vectors folded into the
    projection weights, so they come straight out of the matmul). The table
    is stored ROTATED per core: local row r holds global node
    (p*6272 + r) mod 50176, so each core's own dst rows are local rows
    0..6271 with core-independent addressing. PSUM->bf16 copies alternate
    between the vector and scalar engines (both idle during phase A).
  * Edges (no self loops) are bucketed per (dst-block, src-parity) and
    gathered per cell via gpsimd dma_gather (512B rows, superrow int16
    indices), rotated across the 4 SWDGE queues so descriptor generation
    runs on all four Q7 cpu pairs concurrently (~4x). Index padding is -1:
    the Q7 ucode trims trailing negatives, so each core gathers only its
    true edge count (cell sizes are padded to the max over cores).
  * One-hot matrices (dst scatter + transpose) are precomputed on the host
    and DMA-streamed per cell, keeping the vector engine free.
  * Per cell: agg += onehot.T @ [w*xp[src] | w] accumulates messages and the
    softmax denominator per block in PSUM; w = exp(leaky(a_src + a_dst)),
    a_src from head-wise row sums of the pre-scaled gathered rows (2x-mode
    tree adds), a_dst via onehot-transpose matmul against the block sidecar.
  * Self loops: the block's own rows + sidecar give msg_self, added at
    finalize. Finalize: normalize, transpose, fused BN+bias (att_src
    unscale folded into BN scale) + ReLU, final linear -> [6272, 64].
"""

import numpy as np
import ml_dtypes

BF16 = ml_dtypes.bfloat16

N, E, F, H, C = 50000, 800000, 256, 4, 64
NEG_SLOPE = 0.2
BN_EPS = 1e-5
NCORES = 8
BLK = 128
NB = 49
OWN = NB * BLK           # 6272
NPAD = NCORES * OWN      # 50176
NT = NPAD // 128         # 392
NTC = NT // 4            # 98 chunks of 4 tiles

# feature permutation: new index j = c*4 + h  <->  old index h*64 + c
_OLD_OF_NEW = (np.arange(F) % H) * C + (np.arange(F) // H)

LAST_EXEC_NS = None
LAST_RESULTS = None


def _prep_edges(edge_index):
    src = np.asarray(edge_index[0], dtype=np.int64)
    dst = np.asarray(edge_index[1], dtype=np.int64)

    core = dst // OWN
    dst_local = dst - core * OWN
    block = dst_local // BLK
    dst_slot = (dst_local % BLK).astype(np.float32)    # identity slot map
    # rotated local row of src on the owning core; parity preserved
    rot = (src[None, :] - (np.arange(NCORES) * OWN)[:, None]) % NPAD  # [8, E]
    cls = (src % 2).astype(np.int64)
    gidx_rot = rot // 2                                # [8, E] superrow/core

    ncell = NB * 2
    cell = core * ncell + block * 2 + cls
    counts = np.bincount(cell, minlength=NCORES * ncell).reshape(NCORES, ncell)
    nie_list = [int(np.ceil(counts[:, ci].max() / 16)) * 16
                for ci in range(ncell)]
    mn_list = [int(counts[:, ci].min()) for ci in range(ncell)]
    subt_list = [(n + 127) // 128 for n in nie_list]
    ni_list = [s * 128 for s in subt_list]
    nie_list = ni_list
    offs = np.zeros(ncell + 1, dtype=np.int64)
    np.cumsum(ni_list, out=offs[1:])
    TOT = int(offs[-1])

    order = np.argsort(cell, kind="stable")
    sorted_cell = cell[order]
    cell_starts = np.zeros(NCORES * ncell + 1, dtype=np.int64)
    np.cumsum(counts.reshape(-1), out=cell_starts[1:])
    rank = np.arange(len(order)) - cell_starts[sorted_cell]
    ci_of = sorted_cell % ncell
    core_of = sorted_cell // ncell
    flat_pos = offs[ci_of] + rank                      # position within core

    gidx_pad = np.zeros((NCORES, TOT), dtype=np.int64)
    gidx_pad[core_of, flat_pos] = gidx_rot[core_of, order]
    dstm_pad = np.full((NCORES, TOT), 200.0, dtype=np.float32)
    dstm_pad[core_of, flat_pos] = dst_slot[order]

    # wrapped gather indices [16, TOT//16] -> replicated x8 across partitions
    g = gidx_pad.astype(np.int16).reshape(NCORES, TOT // 16, 16)
    g = np.ascontiguousarray(g.transpose(0, 2, 1))
    idx_all = np.tile(g, (1, 8, 1))                    # [8, 128, TOT//16]

    # host-built one-hots, bf16:
    #   oh[p, (t,d)]  = (dstm_gather[p, t] == d)   (partition = edge lane)
    #   ohT[d, (t,e)] = (dstm_flat[t*128+e] == d)  (partition = dst slot)
    dst_w = np.empty((NCORES, 128, TOT // 128), dtype=np.float32)
    for ci in range(ncell):
        seg = dstm_pad[:, offs[ci]:offs[ci + 1]].reshape(
            NCORES, subt_list[ci], 128)
        dst_w[:, :, offs[ci] // 128:offs[ci + 1] // 128] = \
            seg.transpose(0, 2, 1)
    dvals = np.arange(128, dtype=np.float32)
    oh_all = np.empty((NCORES, 128, TOT), dtype=BF16)
    ohT_all = np.empty((NCORES, 128, TOT), dtype=BF16)
    for p in range(NCORES):
        oh = (dst_w[p][:, :, None] == dvals).astype(BF16)   # [128, T/128, 128]
        oh_all[p] = oh.reshape(128, TOT)
        ohT_all[p] = (dvals[:, None] == dstm_pad[p][None, :]).astype(BF16)

    return idx_all, oh_all, ohT_all, (subt_list, nie_list, mn_list)


def _prep_params(x, W, att_src, att_dst, gat_bias, bn_gamma, bn_beta,
                 bn_mean, bn_var, lin_W, lin_b):
    f32 = np.float32
    W = np.asarray(W, f32)
    att_src_f = np.asarray(att_src, f32).reshape(H * C)      # index h*64+c
    att_src_hc = np.asarray(att_src, f32)                    # [H, C]
    att_dst_hc = np.asarray(att_dst, f32)

    wt = W.T                                                 # [in, out_old]
    wt_perm = wt[:, _OLD_OF_NEW] * att_src_f[_OLD_OF_NEW][None, :]
    aw_src = np.zeros((F, H), dtype=f32)
    aw_dst = np.zeros((F, H), dtype=f32)
    for h in range(H):
        aw_src[:, h] = W[h * C:(h + 1) * C, :].T @ att_src_hc[h]
        aw_dst[:, h] = W[h * C:(h + 1) * C, :].T @ att_dst_hc[h]
    wt_full = np.concatenate([wt_perm, aw_src, aw_dst], axis=1)  # [256, 264]
    wt_ext = np.ascontiguousarray(wt_full.reshape(2, 128, 264)).astype(BF16)

    xT = np.zeros((F, NPAD), dtype=f32)
    xT[:, :N] = np.asarray(x, f32).T
    xT_t = np.ascontiguousarray(
        xT.reshape(2, 128, NT, 128).transpose(2, 1, 0, 3)).astype(BF16)

    bnscale = np.asarray(bn_gamma, f32) / np.sqrt(np.asarray(bn_var, f32) + BN_EPS)
    bnshift = ((np.asarray(gat_bias, f32) - np.asarray(bn_mean, f32)) * bnscale
               + np.asarray(bn_beta, f32))
    bnsc_f = bnscale[_OLD_OF_NEW] / att_src_f[_OLD_OF_NEW]   # fold unscale
    bnsc = np.ascontiguousarray(bnsc_f.reshape(2, 128).T)
    bnsh = np.ascontiguousarray(bnshift[_OLD_OF_NEW].reshape(2, 128).T)

    linw = np.asarray(lin_W, f32).T[_OLD_OF_NEW, :]
    linw_t = np.ascontiguousarray(linw.reshape(2, 128, 64)).astype(BF16)
    linb_rep = np.tile(np.asarray(lin_b, f32)[None, :], (128, 1))

    ident_f32 = np.eye(128, dtype=np.float32)

    return dict(xT_t=xT_t, wt_ext=wt_ext, bnsc=bnsc.astype(f32),
                bnsh=bnsh.astype(f32), linw=linw_t, linb=linb_rep.astype(f32),
                ident_f32=ident_f32)


def _build(subt_cfg, queue_map=None):
    import concourse.bacc as bacc
    import concourse.mybir as mybir
    import concourse.tile as tile

    dt = mybir.dt
    subt_list, nie_list, mn_list = subt_cfg
    ni_list = [s * 128 for s in subt_list]
    offs = [0]
    for n in ni_list:
        offs.append(offs[-1] + n)
    TOT = offs[-1]
    SMAX = max(subt_list)

    nc = bacc.Bacc("TRN2", target_bir_lowering=False, debug=False,
                   enable_asserts=False, num_devices=NCORES,
                   num_swdge_queues=4)

    xT4_in = nc.dram_tensor("xT4", [NTC, 128, 4, 2, 128], dt.bfloat16,
                            kind="ExternalInput")
    wt_in = nc.dram_tensor("wt_ext", [2, 128, 264], dt.bfloat16,
                           kind="ExternalInput")
    bnsc_in = nc.dram_tensor("bnsc", [128, 2], dt.float32, kind="ExternalInput")
    bnsh_in = nc.dram_tensor("bnsh", [128, 2], dt.float32, kind="ExternalInput")
    linw_in = nc.dram_tensor("linw", [2, 128, 64], dt.bfloat16, kind="ExternalInput")
    linb_in = nc.dram_tensor("linb", [128, 64], dt.float32, kind="ExternalInput")
    identf_in = nc.dram_tensor("ident_f32", [128, 128], dt.float32, kind="ExternalInput")
    idx_in = nc.dram_tensor("idx", [128, TOT // 16], dt.int16, kind="ExternalInput")
    oh_in = nc.dram_tensor("oh", [128, TOT], dt.bfloat16, kind="ExternalInput")
    ohT_in = nc.dram_tensor("ohT", [128, TOT], dt.bfloat16, kind="ExternalInput")
    out_dram = nc.dram_tensor("out", [OWN, 64], dt.float32, kind="ExternalOutput")

    with tile.TileContext(nc) as tc:
        with (
            tc.tile_pool(name="dram", bufs=1, space="DRAM") as dramp,
            tc.tile_pool(name="const", bufs=1) as constp,
        ):
            xp_tab = dramp.tile([NPAD, 256], dt.bfloat16)
            att_tab = dramp.tile([NPAD, 8], dt.bfloat16)
            sup = xp_tab[:].rearrange("(s two) f -> s (two f)", two=2)
            tabw = xp_tab[:].rearrange("(c j p) f -> c p j f", j=4, p=128)
            attw = att_tab[:].rearrange("(c j p) f -> c p j f", j=4, p=128)

            # ---- consts ----
            wt_sb = constp.tile([128, 2, 264], dt.bfloat16)
            for k in range(2):
                nc.sync.dma_start(out=wt_sb[:, k, :], in_=wt_in[k])
            idx_sb = constp.tile([128, TOT // 16], dt.int16)
            nc.sync.dma_start(out=idx_sb[:], in_=idx_in[:])
            bnsc_sb = constp.tile([128, 2], dt.float32)
            nc.sync.dma_start(out=bnsc_sb[:], in_=bnsc_in[:])
            bnsh_sb = constp.tile([128, 2], dt.float32)
            nc.sync.dma_start(out=bnsh_sb[:], in_=bnsh_in[:])
            linw_sb = constp.tile([128, 2, 64], dt.bfloat16)
            for k in range(2):
                nc.sync.dma_start(out=linw_sb[:, k, :], in_=linw_in[k])
            linb_sb = constp.tile([128, 64], dt.float32)
            nc.sync.dma_start(out=linb_sb[:], in_=linb_in[:])
            identf_sb = constp.tile([128, 128], dt.float32)
            nc.sync.dma_start(out=identf_sb[:], in_=identf_in[:])

            # ---- phase A: replicated projection, 4-tile chunks ----
            with (
                tc.tile_pool(name="proj_sb", bufs=8) as psb,
                tc.tile_pool(name="proj_out", bufs=8) as pxp,
                tc.tile_pool(name="proj_ps", bufs=8, space="PSUM") as pps,
            ):
                for c in range(NTC):
                    xt = psb.tile([128, 4, 2, 128], dt.bfloat16)
                    nc.sync.dma_start(out=xt[:], in_=xT4_in[c])
                    xp4 = pxp.tile([128, 4, 264], dt.bfloat16)
                    for j in range(4):
                        ps = pps.tile([128, 264], dt.float32, space="PSUM")
                        nc.tensor.matmul(out=ps[:], lhsT=xt[:, j, 0, :],
                                         rhs=wt_sb[:, 0, :],
                                         start=True, stop=False)
                        nc.tensor.matmul(out=ps[:], lhsT=xt[:, j, 1, :],
                                         rhs=wt_sb[:, 1, :],
                                         start=False, stop=True)
                        if j % 2 == 0:
                            nc.scalar.activation(
                                xp4[:, j, :], ps[:],
                                mybir.ActivationFunctionType.Copy)
                        else:
                            nc.vector.tensor_copy(out=xp4[:, j, :], in_=ps[:])
                    nc.sync.dma_start(out=tabw[c], in_=xp4[:, :, 0:256])
                    nc.sync.dma_start(out=attw[c], in_=xp4[:, :, 256:264])

            # ---- phase B: per-block pipeline ----
            with (
                tc.tile_pool(name="gsb", bufs=4) as gsb,
                tc.tile_pool(name="ohsb", bufs=10) as ohsb,
                tc.tile_pool(name="msb", bufs=4) as msb,
                tc.tile_pool(name="osb", bufs=3) as osb,
                tc.tile_pool(name="fsb", bufs=2) as fsb,
                tc.tile_pool(name="aggps", bufs=3, space="PSUM") as aggps,
                tc.tile_pool(name="adstps", bufs=2, space="PSUM") as adstps,
                tc.tile_pool(name="tps", bufs=2, space="PSUM") as tps,
                tc.tile_pool(name="finps", bufs=1, space="PSUM") as finps,
            ):
                qctr = [0]
                gather_insts = []

                def gq():
                    i = qctr[0]
                    qctr[0] += 1
                    return queue_map[i] if queue_map is not None else 0

                for b in range(NB):
                    own_x = osb.tile([128, 256], dt.bfloat16, tag="ox")
                    nc.sync.dma_start(
                        out=own_x[:], in_=xp_tab[b * 128:(b + 1) * 128, :])
                    own_a = osb.tile([128, 8], dt.bfloat16, tag="oa")
                    nc.sync.dma_start(
                        out=own_a[:], in_=att_tab[b * 128:(b + 1) * 128, :])

                    agg = aggps.tile([128, 260], dt.float32, space="PSUM")
                    for cls in range(2):
                        ci = b * 2 + cls
                        S = subt_list[ci]
                        NI = ni_list[ci]
                        oE = offs[ci]
                        oW = oE // 16
                        nie = nie_list[ci]
                        xg = gsb.tile([128, SMAX, 256], dt.bfloat16,
                                      tag=f"xg{cls}")
                        src_ap = sup[:, 0:256] if cls == 0 else sup[:, 256:512]
                        for g0 in range(0, S, 8):
                            nrem = min(nie - g0 * 128, 1024)
                            if nrem <= 0:
                                break
                            gs = (nrem + 127) // 128
                            gi = nc.gpsimd.dma_gather(
                                out_ap=xg[:, g0:g0 + gs, :], in_ap=src_ap,
                                idxs_ap=idx_sb[:, oW + g0 * 8:
                                               oW + g0 * 8 + (nrem + 15) // 16],
                                num_idxs=nrem, num_idxs_reg=nrem,
                                elem_size=256, elem_step=512, queue_num=gq())
                            gather_insts.append(gi)
                        # host-precomputed one-hots
                        oh = ohsb.tile([128, SMAX, 128], dt.bfloat16, tag="oh")
                        nc.scalar.dma_start(
                            out=oh[:, 0:S, :],
                            in_=oh_in[:, oE:oE + NI].rearrange(
                                "p (t d) -> p t d", d=128))
                        ohT = ohsb.tile([128, SMAX, 128], dt.bfloat16, tag="ohT")
                        nc.scalar.dma_start(
                            out=ohT[:, 0:S, :],
                            in_=ohT_in[:, oE:oE + NI].rearrange(
                                "p (t e) -> p t e", e=128))
                        # a_src: head-wise row sums via 2x-mode tree adds
                        xg4 = xg[:, 0:S, :].rearrange(
                            "p t (c h) -> p t c h", h=H)
                        tr1 = msb.tile([128, SMAX, 32, 4], dt.bfloat16,
                                       tag="tr1")
                        nc.vector.tensor_tensor(
                            out=tr1[:, 0:S, :, :], in0=xg4[:, :, 0:32, :],
                            in1=xg4[:, :, 32:64, :], op=mybir.AluOpType.add)
                        tr2 = msb.tile([128, SMAX, 16, 4], dt.bfloat16,
                                       tag="tr2")
                        nc.vector.tensor_tensor(
                            out=tr2[:, 0:S, :, :], in0=tr1[:, 0:S, 0:16, :],
                            in1=tr1[:, 0:S, 16:32, :], op=mybir.AluOpType.add)
                        tr3 = msb.tile([128, SMAX, 8, 4], dt.bfloat16,
                                       tag="tr3")
                        nc.vector.tensor_tensor(
                            out=tr3[:, 0:S, :, :], in0=tr2[:, 0:S, 0:8, :],
                            in1=tr2[:, 0:S, 8:16, :], op=mybir.AluOpType.add)
                        asrc = msb.tile([128, SMAX, 4], dt.float32, tag="asrc")
                        nc.vector.reduce_sum(
                            out=asrc[:, 0:S, :],
                            in_=tr3[:, 0:S, :, :].rearrange(
                                "p t c h -> p t h c"),
                            axis=mybir.AxisListType.X)
                        # a_dst per edge via one-hot-transpose matmuls
                        adps = adstps.tile([128, SMAX, 4], dt.float32,
                                           space="PSUM")
                        for t in range(S):
                            nc.tensor.matmul(out=adps[:, t, :],
                                             lhsT=ohT[:, t, :],
                                             rhs=own_a[:, 4:8],
                                             start=True, stop=True)
                        # w = exp(leaky(a_src + a_dst)) -> msg cols 256:260
                        ev = msb.tile([128, SMAX, 4], dt.float32, tag="ev")
                        nc.vector.tensor_tensor(out=ev[:, 0:S, :],
                                                in0=asrc[:, 0:S, :],
                                                in1=adps[:, 0:S, :],
                                                op=mybir.AluOpType.add)
                        lv = msb.tile([128, SMAX, 4], dt.float32, tag="lv")
                        nc.vector.scalar_tensor_tensor(
                            out=lv[:, 0:S, :], in0=ev[:, 0:S, :],
                            scalar=NEG_SLOPE, in1=ev[:, 0:S, :],
                            op0=mybir.AluOpType.mult,
                            op1=mybir.AluOpType.max)
                        msg = msb.tile([128, SMAX, 260], dt.bfloat16,
                                       tag="msg")
                        nc.scalar.activation(msg[:, 0:S, 256:260],
                                             lv[:, 0:S, :],
                                             mybir.ActivationFunctionType.Exp)
                        nc.vector.tensor_tensor(
                            out=msg[:, 0:S, 0:256].rearrange(
                                "p t (c h) -> p t c h", h=H),
                            in0=xg4[:],
                            in1=msg[:, 0:S, 256:260][:, :, None, :]
                                .to_broadcast([128, S, C, H]),
                            op=mybir.AluOpType.mult)
                        for t in range(S):
                            nc.tensor.matmul(
                                out=agg[:], lhsT=oh[:, t, :],
                                rhs=msg[:, t, :],
                                start=(cls == 0 and t == 0),
                                stop=(cls == 1 and t == S - 1))
                    # ---- finalize (self loop + normalize + BN + linear) ----
                    evs = fsb.tile([128, 4], dt.float32, tag="evs")
                    nc.vector.tensor_tensor(out=evs[:], in0=own_a[:, 0:4],
                                            in1=own_a[:, 4:8],
                                            op=mybir.AluOpType.add)
                    lvs = fsb.tile([128, 4], dt.float32, tag="lvs")
                    nc.vector.scalar_tensor_tensor(
                        out=lvs[:], in0=evs[:], scalar=NEG_SLOPE, in1=evs[:],
                        op0=mybir.AluOpType.mult, op1=mybir.AluOpType.max)
                    selfmsg = fsb.tile([128, 260], dt.float32, tag="sm")
                    nc.scalar.activation(selfmsg[:, 256:260], lvs[:],
                                         mybir.ActivationFunctionType.Exp)
                    nc.vector.tensor_tensor(
                        out=selfmsg[:, 0:256].rearrange(
                            "p (c h) -> p c h", h=H),
                        in0=own_x[:].rearrange("p (c h) -> p c h", h=H),
                        in1=selfmsg[:, 256:260][:, None, :].to_broadcast(
                            [128, C, H]),
                        op=mybir.AluOpType.mult)
                    tot = fsb.tile([128, 260], dt.float32, tag="tot")
                    nc.vector.tensor_tensor(out=tot[:], in0=agg[:],
                                            in1=selfmsg[:],
                                            op=mybir.AluOpType.add)
                    rec = fsb.tile([128, 4], dt.float32, tag="rec")
                    nc.vector.reciprocal(rec[:], tot[:, 256:260])
                    gat = fsb.tile([128, 256], dt.float32, tag="gat")
                    nc.vector.tensor_tensor(
                        out=gat[:].rearrange("p (c h) -> p c h", h=H),
                        in0=tot[:, 0:256].rearrange("p (c h) -> p c h", h=H),
                        in1=rec[:, None, :].to_broadcast([128, C, H]),
                        op=mybir.AluOpType.mult)
                    fps = finps.tile([128, 64], dt.float32, space="PSUM")
                    gt = fsb.tile([128, 2, 128], dt.bfloat16, tag="gt")
                    for k in range(2):
                        pst = tps.tile([128, 128], dt.float32, space="PSUM",
                                       tag="pst")
                        nc.tensor.transpose(out=pst[:],
                                            in_=gat[:, k * 128:(k + 1) * 128],
                                            identity=identf_sb[:])
                        nc.scalar.activation(gt[:, k, :], pst[:],
                                             mybir.ActivationFunctionType.Relu,
                                             bias=bnsh_sb[:, k:k + 1],
                                             scale=bnsc_sb[:, k:k + 1])
                        nc.tensor.matmul(out=fps[:], lhsT=gt[:, k, :],
                                         rhs=linw_sb[:, k, :],
                                         start=(k == 0), stop=(k == 1))
                    ob = fsb.tile([128, 64], dt.float32, tag="ob")
                    nc.vector.tensor_tensor(out=ob[:], in0=fps[:],
                                            in1=linb_sb[:],
                                            op=mybir.AluOpType.add)
                    nc.sync.dma_start(
                        out=out_dram[b * 128:(b + 1) * 128, :], in_=ob[:])
    nc.compile()
    return nc, gather_insts


def _queue_map_from_lanes(gather_insts):
    """Pass-1 lane readback: queue k must equal (DMASW lane) % 4."""
    from concourse.tile_scheduler import PROC_NAMES
    qmap = []
    for gi in gather_insts:
        name = PROC_NAMES[gi.ins.bass_scheduled_proc]
        assert name.startswith("DMASW"), name
        qmap.append(int(name[5:]) % 4)
    return qmap


def _install_ntff_shim():
    """Install the axon NTFF profiling hook (missing antenv.axon_hooks shim)."""
    import sys, types
    if "antenv.axon_hooks" in sys.modules:
        return
    m = types.ModuleType("antenv.axon_hooks")
    _h = [None]
    m.set_axon_ntff_profile_hook = lambda h: _h.__setitem__(0, h)
    m.get_axon_ntff_profile_hook = lambda: _h[0]
    sys.modules["antenv.axon_hooks"] = m
    import antenv
    antenv.axon_hooks = m
    from trn_agent_boot.trn_boot import _ntff_profile_via_ctypes
    hook = _ntff_profile_via_ctypes("/opt/axon/libaxon_pjrt.so")
    if hook is not None:
        m.set_axon_ntff_profile_hook(hook)


def kernel(**inputs):
    global LAST_EXEC_NS, LAST_RESULTS
    import os
    from concourse import bass_utils

    trace = os.environ.get("KERNEL_TRACE") == "1"
    if trace:
        try:
            _install_ntff_shim()
            bass_utils.upload_artifacts = lambda tmpdir: "(upload skipped)"
        except Exception as e:
            print("ntff shim failed:", e)
            trace = False

    idx_all, oh_all, ohT_all, subt_cfg = _prep_edges(
        np.asarray(inputs["edge_index"]))
    params = _prep_params(
        inputs["x"], inputs["W"], inputs["att_src"], inputs["att_dst"],
        inputs["gat_bias"], inputs["bn_gamma"], inputs["bn_beta"],
        inputs["bn_mean"], inputs["bn_var"], inputs["lin_W"], inputs["lin_b"])

    nc1, ginsts = _build(subt_cfg)
    nc, _ = _build(subt_cfg, queue_map=_queue_map_from_lanes(ginsts))

    xT_t = params["xT_t"]                    # [NT, 128, 2, 128]
    shared = dict(
        wt_ext=params["wt_ext"], bnsc=params["bnsc"], bnsh=params["bnsh"],
        linw=params["linw"], linb=params["linb"],
        ident_f32=params["ident_f32"])
    in_maps = []
    for p in range(NCORES):
        m = dict(shared)
        rot = np.roll(np.arange(NT), -p * NB)     # tile t holds local rows
        xr = xT_t[rot]
        m["xT4"] = np.ascontiguousarray(
            xr.reshape(NTC, 4, 128, 2, 128).transpose(0, 2, 1, 3, 4))
        m["idx"] = np.ascontiguousarray(idx_all[p])
        m["oh"] = np.ascontiguousarray(oh_all[p])
        m["ohT"] = np.ascontiguousarray(ohT_all[p])
        in_maps.append(m)

    run_kwargs = {}
    if trace:
        run_kwargs = dict(trace=True, tmpdir=os.environ.get(
            "KERNEL_TRACE_DIR", "/tmp/gat_prof"))
        os.makedirs(run_kwargs["tmpdir"], exist_ok=True)
    res = bass_utils.run_bass_kernel_spmd(
        nc, in_maps, core_ids=list(range(NCORES)), **run_kwargs)
    LAST_EXEC_NS = res.exec_time_ns
    LAST_RESULTS = res

    full = np.empty((NPAD, 64), dtype=np.float32)
    for p in range(NCORES):
        full[p * OWN:(p + 1) * OWN] = res.results[p]["out"]
    return full[:N]


# revision 22
# speedup vs baseline: 1.7519x; 1.1050x over previous
"""GAT layer (gnn_message_passing) on 8 Trainium2 NeuronCores — V5.

Strategy (dst-partitioned, replicated projection into rotated local tables):
  * Core p owns dst nodes [p*6272, (p+1)*6272) = 49 blocks of 128.
  * Every core computes the full projected table xp = x @ W.T (bf16,
    feature-permuted j = c*4+h, pre-scaled by att_src) plus an 8-col
    attention sidecar (a_src/a_dst per node — att vectors folded into the
    projection weights, so they come straight out of the matmul). The table
    is stored ROTATED per core: local row r holds global node
    (p*6272 + r) mod 50176, so each core's own dst rows are local rows
    0..6271 with core-independent addressing. PSUM->bf16 copies alternate
    between the vector and scalar engines (both idle during phase A).
  * Edges (no self loops) are bucketed per (dst-block, src-parity) and
    gathered per cell via gpsimd dma_gather (512B rows, superrow int16
    indices), rotated across the 4 SWDGE queues so descriptor generation
    runs on all four Q7 cpu pairs concurrently (~4x). Index padding is -1:
    the Q7 ucode trims trailing negatives, so each core gathers only its
    true edge count (cell sizes are padded to the max over cores).
  * One-hot matrices (dst scatter + transpose) are precomputed on the host
    and DMA-streamed per cell, keeping the vector engine free.
  * Per cell: agg += onehot.T @ [w*xp[src] | w] accumulates messages and the
    softmax denominator per block in PSUM; w = exp(leaky(a_src + a_dst)),
    a_src from head-wise row sums of the pre-scaled gathered rows (2x-mode
    tree adds), a_dst via onehot-transpose matmul against the block sidecar.
  * Self loops: the block's own rows + sidecar give msg_self, added at
    finalize. Finalize: normalize, transpose, fused BN+bias (att_src
    unscale folded into BN scale) + ReLU, final linear -> [6272, 64].
"""

import numpy as np
import ml_dtypes

BF16 = ml_dtypes.bfloat16

N, E, F, H, C = 50000, 800000, 256, 4, 64
NEG_SLOPE = 0.2
BN_EPS = 1e-5
NCORES = 8
BLK = 128
NB = 49
OWN = NB * BLK           # 6272
NPAD = NCORES * OWN      # 50176
NT = NPAD // 128         # 392
NTC = NT // 4            # 98 chunks of 4 tiles

# feature permutation: new index j = c*4 + h  <->  old index h*64 + c
_OLD_OF_NEW = (np.arange(F) % H) * C + (np.arange(F) // H)

LAST_EXEC_NS = None
LAST_RESULTS = None


def _prep_edges(edge_index):
    src = np.asarray(edge_index[0], dtype=np.int64)
    dst = np.asarray(edge_index[1], dtype=np.int64)

    core = dst // OWN
    dst_local = dst - core * OWN
    block = dst_local // BLK
    dst_slot = (dst_local % BLK).astype(np.float32)    # identity slot map
    # partition-major table position on the owning core (rotated tiles):
    # node at (q = src%128, tile_rot) -> pm row q*NT + tile_rot
    q = src % 128
    tile_rot = (src // 128 - core * NB) % NT
    cls = (tile_rot % 2).astype(np.int64)
    pm = q * NT + tile_rot
    gidx = (pm // 2).astype(np.int64)                  # table superrow

    ncell = NB * 2
    cell = core * ncell + block * 2 + cls
    counts = np.bincount(cell, minlength=NCORES * ncell).reshape(NCORES, ncell)
    nie_list = [int(np.ceil(counts[:, ci].max() / 16)) * 16
                for ci in range(ncell)]
    mn_list = [int(counts[:, ci].min()) for ci in range(ncell)]
    subt_list = [(n + 127) // 128 for n in nie_list]
    ni_list = [s * 128 for s in subt_list]
    nie_list = ni_list
    offs = np.zeros(ncell + 1, dtype=np.int64)
    np.cumsum(ni_list, out=offs[1:])
    TOT = int(offs[-1])

    order = np.argsort(cell, kind="stable")
    sorted_cell = cell[order]
    cell_starts = np.zeros(NCORES * ncell + 1, dtype=np.int64)
    np.cumsum(counts.reshape(-1), out=cell_starts[1:])
    rank = np.arange(len(order)) - cell_starts[sorted_cell]
    ci_of = sorted_cell % ncell
    core_of = sorted_cell // ncell
    flat_pos = offs[ci_of] + rank                      # position within core

    gidx_pad = np.zeros((NCORES, TOT), dtype=np.int64)
    gidx_pad[core_of, flat_pos] = gidx[order]
    dstm_pad = np.full((NCORES, TOT), 200.0, dtype=np.float32)
    dstm_pad[core_of, flat_pos] = dst_slot[order]

    # wrapped gather indices [16, TOT//16] -> replicated x8 across partitions
    g = gidx_pad.astype(np.int16).reshape(NCORES, TOT // 16, 16)
    g = np.ascontiguousarray(g.transpose(0, 2, 1))
    idx_all = np.tile(g, (1, 8, 1))                    # [8, 128, TOT//16]

    # host-built one-hots, bf16:
    #   oh[p, (t,d)]  = (dstm_gather[p, t] == d)   (partition = edge lane)
    #   ohT[d, (t,e)] = (dstm_flat[t*128+e] == d)  (partition = dst slot)
    dst_w = np.empty((NCORES, 128, TOT // 128), dtype=np.float32)
    for ci in range(ncell):
        seg = dstm_pad[:, offs[ci]:offs[ci + 1]].reshape(
            NCORES, subt_list[ci], 128)
        dst_w[:, :, offs[ci] // 128:offs[ci + 1] // 128] = \
            seg.transpose(0, 2, 1)
    dvals = np.arange(128, dtype=np.float32)
    oh_all = np.empty((NCORES, 128, TOT), dtype=BF16)
    ohT_all = np.empty((NCORES, 128, TOT), dtype=BF16)
    for p in range(NCORES):
        oh = (dst_w[p][:, :, None] == dvals).astype(BF16)   # [128, T/128, 128]
        oh_all[p] = oh.reshape(128, TOT)
        ohT_all[p] = (dvals[:, None] == dstm_pad[p][None, :]).astype(BF16)

    return idx_all, oh_all, ohT_all, (subt_list, nie_list, mn_list)


def _prep_params(x, W, att_src, att_dst, gat_bias, bn_gamma, bn_beta,
                 bn_mean, bn_var, lin_W, lin_b):
    f32 = np.float32
    W = np.asarray(W, f32)
    att_src_f = np.asarray(att_src, f32).reshape(H * C)      # index h*64+c
    att_src_hc = np.asarray(att_src, f32)                    # [H, C]
    att_dst_hc = np.asarray(att_dst, f32)

    wt = W.T                                                 # [in, out_old]
    wt_perm = wt[:, _OLD_OF_NEW] * att_src_f[_OLD_OF_NEW][None, :]
    aw_src = np.zeros((F, H), dtype=f32)
    aw_dst = np.zeros((F, H), dtype=f32)
    for h in range(H):
        aw_src[:, h] = W[h * C:(h + 1) * C, :].T @ att_src_hc[h]
        aw_dst[:, h] = W[h * C:(h + 1) * C, :].T @ att_dst_hc[h]
    wt_full = np.concatenate([wt_perm, aw_src, aw_dst], axis=1)  # [256, 264]
    wt_ext = np.ascontiguousarray(wt_full.reshape(2, 128, 264)).astype(BF16)

    xT = np.zeros((F, NPAD), dtype=f32)
    xT[:, :N] = np.asarray(x, f32).T
    xT_t = np.ascontiguousarray(
        xT.reshape(2, 128, NT, 128).transpose(2, 1, 0, 3)).astype(BF16)

    bnscale = np.asarray(bn_gamma, f32) / np.sqrt(np.asarray(bn_var, f32) + BN_EPS)
    bnshift = ((np.asarray(gat_bias, f32) - np.asarray(bn_mean, f32)) * bnscale
               + np.asarray(bn_beta, f32))
    bnsc_f = bnscale[_OLD_OF_NEW] / att_src_f[_OLD_OF_NEW]   # fold unscale
    bnsc = np.ascontiguousarray(bnsc_f.reshape(2, 128).T)
    bnsh = np.ascontiguousarray(bnshift[_OLD_OF_NEW].reshape(2, 128).T)

    linw = np.asarray(lin_W, f32).T[_OLD_OF_NEW, :]
    linw_t = np.ascontiguousarray(linw.reshape(2, 128, 64)).astype(BF16)
    linb_rep = np.tile(np.asarray(lin_b, f32)[None, :], (128, 1))

    ident_f32 = np.eye(128, dtype=np.float32)

    return dict(xT_t=xT_t, wt_ext=wt_ext, bnsc=bnsc.astype(f32),
                bnsh=bnsh.astype(f32), linw=linw_t, linb=linb_rep.astype(f32),
                ident_f32=ident_f32)


def _build(subt_cfg, queue_map=None):
    import concourse.bacc as bacc
    import concourse.mybir as mybir
    import concourse.tile as tile

    dt = mybir.dt
    subt_list, nie_list, mn_list = subt_cfg
    ni_list = [s * 128 for s in subt_list]
    offs = [0]
    for n in ni_list:
        offs.append(offs[-1] + n)
    TOT = offs[-1]
    SMAX = max(subt_list)

    nc = bacc.Bacc("TRN2", target_bir_lowering=False, debug=False,
                   enable_asserts=False, num_devices=NCORES,
                   num_swdge_queues=4)

    xT4_in = nc.dram_tensor("xT4", [NTC, 128, 4, 2, 128], dt.bfloat16,
                            kind="ExternalInput")
    wt_in = nc.dram_tensor("wt_ext", [2, 128, 264], dt.bfloat16,
                           kind="ExternalInput")
    bnsc_in = nc.dram_tensor("bnsc", [128, 2], dt.float32, kind="ExternalInput")
    bnsh_in = nc.dram_tensor("bnsh", [128, 2], dt.float32, kind="ExternalInput")
    linw_in = nc.dram_tensor("linw", [2, 128, 64], dt.bfloat16, kind="ExternalInput")
    linb_in = nc.dram_tensor("linb", [128, 64], dt.float32, kind="ExternalInput")
    identf_in = nc.dram_tensor("ident_f32", [128, 128], dt.float32, kind="ExternalInput")
    idx_in = nc.dram_tensor("idx", [128, TOT // 16], dt.int16, kind="ExternalInput")
    oh_in = nc.dram_tensor("oh", [128, TOT], dt.bfloat16, kind="ExternalInput")
    ohT_in = nc.dram_tensor("ohT", [128, TOT], dt.bfloat16, kind="ExternalInput")
    out_dram = nc.dram_tensor("out", [OWN, 64], dt.float32, kind="ExternalOutput")

    with tile.TileContext(nc) as tc:
        with (
            tc.tile_pool(name="dram", bufs=1, space="DRAM") as dramp,
            tc.tile_pool(name="const", bufs=1) as constp,
        ):
            xp_tab = dramp.tile([NPAD, 256], dt.bfloat16)
            sup = xp_tab[:].rearrange("(s two) f -> s (two f)", two=2)
            # PM row p*NT + t: write chunks contiguously per partition,
            # read own-block rows as [t, p, f]
            tabw = xp_tab[:].rearrange("(p c j) f -> c p j f", p=128, j=4)
            ownv = xp_tab[:].rearrange("(p t) f -> t p f", p=128)

            # ---- consts ----
            wt_sb = constp.tile([128, 2, 264], dt.bfloat16)
            for k in range(2):
                nc.sync.dma_start(out=wt_sb[:, k, :], in_=wt_in[k])
            idx_sb = constp.tile([128, TOT // 16], dt.int16)
            nc.sync.dma_start(out=idx_sb[:], in_=idx_in[:])
            bnsc_sb = constp.tile([128, 2], dt.float32)
            nc.sync.dma_start(out=bnsc_sb[:], in_=bnsc_in[:])
            bnsh_sb = constp.tile([128, 2], dt.float32)
            nc.sync.dma_start(out=bnsh_sb[:], in_=bnsh_in[:])
            linw_sb = constp.tile([128, 2, 64], dt.bfloat16)
            for k in range(2):
                nc.sync.dma_start(out=linw_sb[:, k, :], in_=linw_in[k])
            linb_sb = constp.tile([128, 64], dt.float32)
            nc.sync.dma_start(out=linb_sb[:], in_=linb_in[:])
            identf_sb = constp.tile([128, 128], dt.float32)
            nc.sync.dma_start(out=identf_sb[:], in_=identf_in[:])
            att_sb = constp.tile([128, NT, 8], dt.bfloat16)

            # ---- phase A: replicated projection, 4-tile chunks ----
            with (
                tc.tile_pool(name="proj_sb", bufs=8) as psb,
                tc.tile_pool(name="proj_out", bufs=8) as pxp,
                tc.tile_pool(name="proj_ps", bufs=8, space="PSUM") as pps,
            ):
                for c in range(NTC):
                    xt = psb.tile([128, 4, 2, 128], dt.bfloat16)
                    nc.sync.dma_start(out=xt[:], in_=xT4_in[c])
                    xp4 = pxp.tile([128, 4, 256], dt.bfloat16)
                    for j in range(4):
                        ps = pps.tile([128, 264], dt.float32, space="PSUM")
                        nc.tensor.matmul(out=ps[:], lhsT=xt[:, j, 0, :],
                                         rhs=wt_sb[:, 0, :],
                                         start=True, stop=False)
                        nc.tensor.matmul(out=ps[:], lhsT=xt[:, j, 1, :],
                                         rhs=wt_sb[:, 1, :],
                                         start=False, stop=True)
                        if j % 2 == 0:
                            nc.scalar.activation(
                                xp4[:, j, :], ps[:, 0:256],
                                mybir.ActivationFunctionType.Copy)
                            nc.vector.tensor_copy(
                                out=att_sb[:, c * 4 + j, :],
                                in_=ps[:, 256:264])
                        else:
                            nc.vector.tensor_copy(out=xp4[:, j, :],
                                                  in_=ps[:, 0:256])
                            nc.scalar.activation(
                                att_sb[:, c * 4 + j, :], ps[:, 256:264],
                                mybir.ActivationFunctionType.Copy)
                    nc.sync.dma_start(out=tabw[c], in_=xp4[:])

            # ---- phase B: per-block pipeline ----
            with (
                tc.tile_pool(name="gsb", bufs=4) as gsb,
                tc.tile_pool(name="ohsb", bufs=10) as ohsb,
                tc.tile_pool(name="msb", bufs=4) as msb,
                tc.tile_pool(name="osb", bufs=3) as osb,
                tc.tile_pool(name="fsb", bufs=2) as fsb,
                tc.tile_pool(name="aggps", bufs=3, space="PSUM") as aggps,
                tc.tile_pool(name="adstps", bufs=2, space="PSUM") as adstps,
                tc.tile_pool(name="tps", bufs=2, space="PSUM") as tps,
                tc.tile_pool(name="finps", bufs=1, space="PSUM") as finps,
            ):
                qctr = [0]
                gather_insts = []

                def gq():
                    i = qctr[0]
                    qctr[0] += 1
                    return queue_map[i] if queue_map is not None else 0

                for b in range(NB):
                    own_x = osb.tile([128, 256], dt.bfloat16, tag="ox")
                    nc.sync.dma_start(out=own_x[:], in_=ownv[b])

                    agg = aggps.tile([128, 260], dt.float32, space="PSUM")
                    for cls in range(2):
                        ci = b * 2 + cls
                        S = subt_list[ci]
                        NI = ni_list[ci]
                        oE = offs[ci]
                        oW = oE // 16
                        nie = nie_list[ci]
                        xg = gsb.tile([128, SMAX, 256], dt.bfloat16,
                                      tag=f"xg{cls}")
                        src_ap = sup[:, 0:256] if cls == 0 else sup[:, 256:512]
                        for g0 in range(0, S, 8):
                            nrem = min(nie - g0 * 128, 1024)
                            if nrem <= 0:
                                break
                            gs = (nrem + 127) // 128
                            gi = nc.gpsimd.dma_gather(
                                out_ap=xg[:, g0:g0 + gs, :], in_ap=src_ap,
                                idxs_ap=idx_sb[:, oW + g0 * 8:
                                               oW + g0 * 8 + (nrem + 15) // 16],
                                num_idxs=nrem, num_idxs_reg=nrem,
                                elem_size=256, elem_step=512, queue_num=gq())
                            gather_insts.append(gi)
                        # host-precomputed one-hots
                        oh = ohsb.tile([128, SMAX, 128], dt.bfloat16, tag="oh")
                        nc.scalar.dma_start(
                            out=oh[:, 0:S, :],
                            in_=oh_in[:, oE:oE + NI].rearrange(
                                "p (t d) -> p t d", d=128))
                        ohT = ohsb.tile([128, SMAX, 128], dt.bfloat16, tag="ohT")
                        nc.scalar.dma_start(
                            out=ohT[:, 0:S, :],
                            in_=ohT_in[:, oE:oE + NI].rearrange(
                                "p (t e) -> p t e", e=128))
                        # a_src: head-wise row sums via 2x-mode tree adds
                        xg4 = xg[:, 0:S, :].rearrange(
                            "p t (c h) -> p t c h", h=H)
                        tr1 = msb.tile([128, SMAX, 32, 4], dt.bfloat16,
                                       tag="tr1")
                        nc.vector.tensor_tensor(
                            out=tr1[:, 0:S, :, :], in0=xg4[:, :, 0:32, :],
                            in1=xg4[:, :, 32:64, :], op=mybir.AluOpType.add)
                        tr2 = msb.tile([128, SMAX, 16, 4], dt.bfloat16,
                                       tag="tr2")
                        nc.vector.tensor_tensor(
                            out=tr2[:, 0:S, :, :], in0=tr1[:, 0:S, 0:16, :],
                            in1=tr1[:, 0:S, 16:32, :], op=mybir.AluOpType.add)
                        tr3 = msb.tile([128, SMAX, 8, 4], dt.bfloat16,
                                       tag="tr3")
                        nc.vector.tensor_tensor(
                            out=tr3[:, 0:S, :, :], in0=tr2[:, 0:S, 0:8, :],
                            in1=tr2[:, 0:S, 8:16, :], op=mybir.AluOpType.add)
                        asrc = msb.tile([128, SMAX, 4], dt.float32, tag="asrc")
                        nc.vector.reduce_sum(
                            out=asrc[:, 0:S, :],
                            in_=tr3[:, 0:S, :, :].rearrange(
                                "p t c h -> p t h c"),
                            axis=mybir.AxisListType.X)
                        # a_dst per edge via one-hot-transpose matmuls
                        adps = adstps.tile([128, SMAX, 4], dt.float32,
                                           space="PSUM")
                        for t in range(S):
                            nc.tensor.matmul(out=adps[:, t, :],
                                             lhsT=ohT[:, t, :],
                                             rhs=att_sb[:, b, 4:8],
                                             start=True, stop=True)
                        # w = exp(leaky(a_src + a_dst)) -> msg cols 256:260
                        ev = msb.tile([128, SMAX, 4], dt.float32, tag="ev")
                        nc.vector.tensor_tensor(out=ev[:, 0:S, :],
                                                in0=asrc[:, 0:S, :],
                                                in1=adps[:, 0:S, :],
                                                op=mybir.AluOpType.add)
                        lv = msb.tile([128, SMAX, 4], dt.float32, tag="lv")
                        nc.vector.scalar_tensor_tensor(
                            out=lv[:, 0:S, :], in0=ev[:, 0:S, :],
                            scalar=NEG_SLOPE, in1=ev[:, 0:S, :],
                            op0=mybir.AluOpType.mult,
                            op1=mybir.AluOpType.max)
                        msg = msb.tile([128, SMAX, 260], dt.bfloat16,
                                       tag="msg")
                        nc.scalar.activation(msg[:, 0:S, 256:260],
                                             lv[:, 0:S, :],
                                             mybir.ActivationFunctionType.Exp)
                        nc.vector.tensor_tensor(
                            out=msg[:, 0:S, 0:256].rearrange(
                                "p t (c h) -> p t c h", h=H),
                            in0=xg4[:],
                            in1=msg[:, 0:S, 256:260][:, :, None, :]
                                .to_broadcast([128, S, C, H]),
                            op=mybir.AluOpType.mult)
                        for t in range(S):
                            nc.tensor.matmul(
                                out=agg[:], lhsT=oh[:, t, :],
                                rhs=msg[:, t, :],
                                start=(cls == 0 and t == 0),
                                stop=(cls == 1 and t == S - 1))
                    # ---- finalize (self loop + normalize + BN + linear) ----
                    evs = fsb.tile([128, 4], dt.float32, tag="evs")
                    nc.vector.tensor_tensor(out=evs[:], in0=att_sb[:, b, 0:4],
                                            in1=att_sb[:, b, 4:8],
                                            op=mybir.AluOpType.add)
                    lvs = fsb.tile([128, 4], dt.float32, tag="lvs")
                    nc.vector.scalar_tensor_tensor(
                        out=lvs[:], in0=evs[:], scalar=NEG_SLOPE, in1=evs[:],
                        op0=mybir.AluOpType.mult, op1=mybir.AluOpType.max)
                    selfmsg = fsb.tile([128, 260], dt.float32, tag="sm")
                    nc.scalar.activation(selfmsg[:, 256:260], lvs[:],
                                         mybir.ActivationFunctionType.Exp)
                    nc.vector.tensor_tensor(
                        out=selfmsg[:, 0:256].rearrange(
                            "p (c h) -> p c h", h=H),
                        in0=own_x[:].rearrange("p (c h) -> p c h", h=H),
                        in1=selfmsg[:, 256:260][:, None, :].to_broadcast(
                            [128, C, H]),
                        op=mybir.AluOpType.mult)
                    tot = fsb.tile([128, 260], dt.float32, tag="tot")
                    nc.vector.tensor_tensor(out=tot[:], in0=agg[:],
                                            in1=selfmsg[:],
                                            op=mybir.AluOpType.add)
                    rec = fsb.tile([128, 4], dt.float32, tag="rec")
                    nc.vector.reciprocal(rec[:], tot[:, 256:260])
                    gat = fsb.tile([128, 256], dt.float32, tag="gat")
                    nc.vector.tensor_tensor(
                        out=gat[:].rearrange("p (c h) -> p c h", h=H),
                        in0=tot[:, 0:256].rearrange("p (c h) -> p c h", h=H),
                        in1=rec[:, None, :].to_broadcast([128, C, H]),
                        op=mybir.AluOpType.mult)
                    fps = finps.tile([128, 64], dt.float32, space="PSUM")
                    gt = fsb.tile([128, 2, 128], dt.bfloat16, tag="gt")
                    for k in range(2):
                        pst = tps.tile([128, 128], dt.float32, space="PSUM",
                                       tag="pst")
                        nc.tensor.transpose(out=pst[:],
                                            in_=gat[:, k * 128:(k + 1) * 128],
                                            identity=identf_sb[:])
                        nc.scalar.activation(gt[:, k, :], pst[:],
                                             mybir.ActivationFunctionType.Relu,
                                             bias=bnsh_sb[:, k:k + 1],
                                             scale=bnsc_sb[:, k:k + 1])
                        nc.tensor.matmul(out=fps[:], lhsT=gt[:, k, :],
                                         rhs=linw_sb[:, k, :],
                                         start=(k == 0), stop=(k == 1))
                    ob = fsb.tile([128, 64], dt.float32, tag="ob")
                    nc.vector.tensor_tensor(out=ob[:], in0=fps[:],
                                            in1=linb_sb[:],
                                            op=mybir.AluOpType.add)
                    nc.sync.dma_start(
                        out=out_dram[b * 128:(b + 1) * 128, :], in_=ob[:])
    nc.compile()
    return nc, gather_insts


def _queue_map_from_lanes(gather_insts):
    """Pass-1 lane readback: queue k must equal (DMASW lane) % 4."""
    from concourse.tile_scheduler import PROC_NAMES
    qmap = []
    for gi in gather_insts:
        name = PROC_NAMES[gi.ins.bass_scheduled_proc]
        assert name.startswith("DMASW"), name
        qmap.append(int(name[5:]) % 4)
    return qmap


def _install_ntff_shim():
    """Install the axon NTFF profiling hook (missing antenv.axon_hooks shim)."""
    import sys, types
    if "antenv.axon_hooks" in sys.modules:
        return
    m = types.ModuleType("antenv.axon_hooks")
    _h = [None]
    m.set_axon_ntff_profile_hook = lambda h: _h.__setitem__(0, h)
    m.get_axon_ntff_profile_hook = lambda: _h[0]
    sys.modules["antenv.axon_hooks"] = m
    import antenv
    antenv.axon_hooks = m
    from trn_agent_boot.trn_boot import _ntff_profile_via_ctypes
    hook = _ntff_profile_via_ctypes("/opt/axon/libaxon_pjrt.so")
    if hook is not None:
        m.set_axon_ntff_profile_hook(hook)


def kernel(**inputs):
    global LAST_EXEC_NS, LAST_RESULTS
    import os
    from concourse import bass_utils

    trace = os.environ.get("KERNEL_TRACE") == "1"
    if trace:
        try:
            _install_ntff_shim()
            bass_utils.upload_artifacts = lambda tmpdir: "(upload skipped)"
        except Exception as e:
            print("ntff shim failed:", e)
            trace = False

    idx_all, oh_all, ohT_all, subt_cfg = _prep_edges(
        np.asarray(inputs["edge_index"]))
    params = _prep_params(
        inputs["x"], inputs["W"], inputs["att_src"], inputs["att_dst"],
        inputs["gat_bias"], inputs["bn_gamma"], inputs["bn_beta"],
        inputs["bn_mean"], inputs["bn_var"], inputs["lin_W"], inputs["lin_b"])

    nc1, ginsts = _build(subt_cfg)
    nc, _ = _build(subt_cfg, queue_map=_queue_map_from_lanes(ginsts))

    xT_t = params["xT_t"]                    # [NT, 128, 2, 128]
    shared = dict(
        wt_ext=params["wt_ext"], bnsc=params["bnsc"], bnsh=params["bnsh"],
        linw=params["linw"], linb=params["linb"],
        ident_f32=params["ident_f32"])
    in_maps = []
    for p in range(NCORES):
        m = dict(shared)
        rot = np.roll(np.arange(NT), -p * NB)     # tile t holds local rows
        xr = xT_t[rot]
        m["xT4"] = np.ascontiguousarray(
            xr.reshape(NTC, 4, 128, 2, 128).transpose(0, 2, 1, 3, 4))
        m["idx"] = np.ascontiguousarray(idx_all[p])
        m["oh"] = np.ascontiguousarray(oh_all[p])
        m["ohT"] = np.ascontiguousarray(ohT_all[p])
        in_maps.append(m)

    run_kwargs = {}
    if trace:
        run_kwargs = dict(trace=True, tmpdir=os.environ.get(
            "KERNEL_TRACE_DIR", "/tmp/gat_prof"))
        os.makedirs(run_kwargs["tmpdir"], exist_ok=True)
    res = bass_utils.run_bass_kernel_spmd(
        nc, in_maps, core_ids=list(range(NCORES)), **run_kwargs)
    LAST_EXEC_NS = res.exec_time_ns
    LAST_RESULTS = res

    full = np.empty((NPAD, 64), dtype=np.float32)
    for p in range(NCORES):
        full[p * OWN:(p + 1) * OWN] = res.results[p]["out"]
    return full[:N]


# revision 23
# speedup vs baseline: 1.8763x; 1.0710x over previous
"""GAT layer (gnn_message_passing) on 8 Trainium2 NeuronCores — V5.

Strategy (dst-partitioned, replicated projection into rotated local tables):
  * Core p owns dst nodes [p*6272, (p+1)*6272) = 49 blocks of 128.
  * Every core computes the full projected table xp = x @ W.T (bf16,
    feature-permuted j = c*4+h, pre-scaled by att_src) plus an 8-col
    attention sidecar (a_src/a_dst per node — att vectors folded into the
    projection weights, so they come straight out of the matmul). The table
    is stored ROTATED per core: local row r holds global node
    (p*6272 + r) mod 50176, so each core's own dst rows are local rows
    0..6271 with core-independent addressing. PSUM->bf16 copies alternate
    between the vector and scalar engines (both idle during phase A).
  * Edges (no self loops) are bucketed per (dst-block, src-parity) and
    gathered per cell via gpsimd dma_gather (512B rows, superrow int16
    indices), rotated across the 4 SWDGE queues so descriptor generation
    runs on all four Q7 cpu pairs concurrently (~4x). Index padding is -1:
    the Q7 ucode trims trailing negatives, so each core gathers only its
    true edge count (cell sizes are padded to the max over cores).
  * One-hot matrices (dst scatter + transpose) are precomputed on the host
    and DMA-streamed per cell, keeping the vector engine free.
  * Per cell: agg += onehot.T @ [w*xp[src] | w] accumulates messages and the
    softmax denominator per block in PSUM; w = exp(leaky(a_src + a_dst)),
    a_src from head-wise row sums of the pre-scaled gathered rows (2x-mode
    tree adds), a_dst via onehot-transpose matmul against the block sidecar.
  * Self loops: the block's own rows + sidecar give msg_self, added at
    finalize. Finalize: normalize, transpose, fused BN+bias (att_src
    unscale folded into BN scale) + ReLU, final linear -> [6272, 64].
"""

import numpy as np
import ml_dtypes

BF16 = ml_dtypes.bfloat16

N, E, F, H, C = 50000, 800000, 256, 4, 64
NEG_SLOPE = 0.2
BN_EPS = 1e-5
NCORES = 8
BLK = 128
NB = 49
OWN = NB * BLK           # 6272
NPAD = NCORES * OWN      # 50176
NT = NPAD // 128         # 392
NTC = NT // 4            # 98 chunks of 4 tiles

# feature permutation: new index j = c*4 + h  <->  old index h*64 + c
_OLD_OF_NEW = (np.arange(F) % H) * C + (np.arange(F) // H)

LAST_EXEC_NS = None
LAST_RESULTS = None


def _prep_edges(edge_index):
    src = np.asarray(edge_index[0], dtype=np.int64)
    dst = np.asarray(edge_index[1], dtype=np.int64)

    core = dst // OWN
    dst_local = dst - core * OWN
    block = dst_local // BLK
    dst_slot = (dst_local % BLK).astype(np.float32)    # identity slot map
    # partition-major table position on the owning core (rotated tiles):
    # node at (q = src%128, tile_rot) -> pm row q*NT + tile_rot
    q = src % 128
    tile_rot = (src // 128 - core * NB) % NT
    cls = (tile_rot % 2).astype(np.int64)
    pm = q * NT + tile_rot
    gidx = (pm // 2).astype(np.int64)                  # table superrow

    ncell = NB * 2
    cell = core * ncell + block * 2 + cls
    counts = np.bincount(cell, minlength=NCORES * ncell).reshape(NCORES, ncell)
    nie_list = [int(np.ceil(counts[:, ci].max() / 16)) * 16
                for ci in range(ncell)]
    mn_list = [int(counts[:, ci].min()) for ci in range(ncell)]
    subt_list = [(n + 127) // 128 for n in nie_list]
    ni_list = [s * 128 for s in subt_list]
    nie_list = ni_list
    offs = np.zeros(ncell + 1, dtype=np.int64)
    np.cumsum(ni_list, out=offs[1:])
    TOT = int(offs[-1])

    order = np.argsort(cell, kind="stable")
    sorted_cell = cell[order]
    cell_starts = np.zeros(NCORES * ncell + 1, dtype=np.int64)
    np.cumsum(counts.reshape(-1), out=cell_starts[1:])
    rank = np.arange(len(order)) - cell_starts[sorted_cell]
    ci_of = sorted_cell % ncell
    core_of = sorted_cell // ncell
    flat_pos = offs[ci_of] + rank                      # position within core

    gidx_pad = np.zeros((NCORES, TOT), dtype=np.int64)
    gidx_pad[core_of, flat_pos] = gidx[order]
    dstm_pad = np.full((NCORES, TOT), 200.0, dtype=np.float32)
    dstm_pad[core_of, flat_pos] = dst_slot[order]

    # wrapped gather indices [16, TOT//16] -> replicated x8 across partitions
    g = gidx_pad.astype(np.int16).reshape(NCORES, TOT // 16, 16)
    g = np.ascontiguousarray(g.transpose(0, 2, 1))
    idx_all = np.tile(g, (1, 8, 1))                    # [8, 128, TOT//16]

    # host-built one-hots, bf16:
    #   oh[p, (t,d)]  = (dstm_gather[p, t] == d)   (partition = edge lane)
    #   ohT[d, (t,e)] = (dstm_flat[t*128+e] == d)  (partition = dst slot)
    dst_w = np.empty((NCORES, 128, TOT // 128), dtype=np.float32)
    for ci in range(ncell):
        seg = dstm_pad[:, offs[ci]:offs[ci + 1]].reshape(
            NCORES, subt_list[ci], 128)
        dst_w[:, :, offs[ci] // 128:offs[ci + 1] // 128] = \
            seg.transpose(0, 2, 1)
    dvals = np.arange(128, dtype=np.float32)
    oh_all = np.empty((NCORES, 128, TOT), dtype=BF16)
    ohT_all = np.empty((NCORES, 128, TOT), dtype=BF16)
    for p in range(NCORES):
        oh = (dst_w[p][:, :, None] == dvals).astype(BF16)   # [128, T/128, 128]
        oh_all[p] = oh.reshape(128, TOT)
        ohT_all[p] = (dvals[:, None] == dstm_pad[p][None, :]).astype(BF16)

    return idx_all, oh_all, ohT_all, (subt_list, nie_list, mn_list)


def _prep_params(x, W, att_src, att_dst, gat_bias, bn_gamma, bn_beta,
                 bn_mean, bn_var, lin_W, lin_b):
    f32 = np.float32
    W = np.asarray(W, f32)
    att_src_f = np.asarray(att_src, f32).reshape(H * C)      # index h*64+c
    att_src_hc = np.asarray(att_src, f32)                    # [H, C]
    att_dst_hc = np.asarray(att_dst, f32)

    wt = W.T                                                 # [in, out_old]
    wt_perm = wt[:, _OLD_OF_NEW] * att_src_f[_OLD_OF_NEW][None, :]
    aw_src = np.zeros((F, H), dtype=f32)
    aw_dst = np.zeros((F, H), dtype=f32)
    for h in range(H):
        aw_src[:, h] = W[h * C:(h + 1) * C, :].T @ att_src_hc[h]
        aw_dst[:, h] = W[h * C:(h + 1) * C, :].T @ att_dst_hc[h]
    wt_full = np.concatenate([wt_perm, aw_src, aw_dst], axis=1)  # [256, 264]
    wt_ext = np.ascontiguousarray(wt_full.reshape(2, 128, 264)).astype(BF16)

    xT = np.zeros((F, NPAD), dtype=f32)
    xT[:, :N] = np.asarray(x, f32).T
    xT_t = np.ascontiguousarray(
        xT.reshape(2, 128, NT, 128).transpose(2, 1, 0, 3)).astype(BF16)

    bnscale = np.asarray(bn_gamma, f32) / np.sqrt(np.asarray(bn_var, f32) + BN_EPS)
    bnshift = ((np.asarray(gat_bias, f32) - np.asarray(bn_mean, f32)) * bnscale
               + np.asarray(bn_beta, f32))
    bnsc_f = bnscale[_OLD_OF_NEW] / att_src_f[_OLD_OF_NEW]   # fold unscale
    bnsc = np.ascontiguousarray(bnsc_f.reshape(2, 128).T)
    bnsh = np.ascontiguousarray(bnshift[_OLD_OF_NEW].reshape(2, 128).T)

    linw = np.asarray(lin_W, f32).T[_OLD_OF_NEW, :]
    linw_t = np.ascontiguousarray(linw.reshape(2, 128, 64)).astype(BF16)
    linb_rep = np.tile(np.asarray(lin_b, f32)[None, :], (128, 1))

    ident_f32 = np.eye(128, dtype=np.float32)

    return dict(xT_t=xT_t, wt_ext=wt_ext, bnsc=bnsc.astype(f32),
                bnsh=bnsh.astype(f32), linw=linw_t, linb=linb_rep.astype(f32),
                ident_f32=ident_f32)


def _build(subt_cfg, queue_map=None):
    import concourse.bacc as bacc
    import concourse.mybir as mybir
    import concourse.tile as tile

    dt = mybir.dt
    subt_list, nie_list, mn_list = subt_cfg
    ni_list = [s * 128 for s in subt_list]
    offs = [0]
    for n in ni_list:
        offs.append(offs[-1] + n)
    TOT = offs[-1]
    SMAX = max(subt_list)

    nc = bacc.Bacc("TRN2", target_bir_lowering=False, debug=False,
                   enable_asserts=False, num_devices=NCORES,
                   num_swdge_queues=4)

    xT4_in = nc.dram_tensor("xT4", [NTC, 128, 4, 2, 128], dt.bfloat16,
                            kind="ExternalInput")
    wt_in = nc.dram_tensor("wt_ext", [2, 128, 264], dt.bfloat16,
                           kind="ExternalInput")
    bnsc_in = nc.dram_tensor("bnsc", [128, 2], dt.float32, kind="ExternalInput")
    bnsh_in = nc.dram_tensor("bnsh", [128, 2], dt.float32, kind="ExternalInput")
    linw_in = nc.dram_tensor("linw", [2, 128, 64], dt.bfloat16, kind="ExternalInput")
    linb_in = nc.dram_tensor("linb", [128, 64], dt.float32, kind="ExternalInput")
    identf_in = nc.dram_tensor("ident_f32", [128, 128], dt.float32, kind="ExternalInput")
    idx_in = nc.dram_tensor("idx", [128, TOT // 16], dt.int16, kind="ExternalInput")
    oh_in = nc.dram_tensor("oh", [128, TOT], dt.bfloat16, kind="ExternalInput")
    ohT_in = nc.dram_tensor("ohT", [128, TOT], dt.bfloat16, kind="ExternalInput")
    out_dram = nc.dram_tensor("out", [OWN, 64], dt.float32, kind="ExternalOutput")

    with tile.TileContext(nc) as tc:
        with (
            tc.tile_pool(name="dram", bufs=1, space="DRAM") as dramp,
            tc.tile_pool(name="const", bufs=1) as constp,
        ):
            xp_tab = dramp.tile([NPAD, 256], dt.bfloat16)
            sup = xp_tab[:].rearrange("(s two) f -> s (two f)", two=2)
            # PM row p*NT + t: write chunks contiguously per partition,
            # read own-block rows as [t, p, f]
            tabw = xp_tab[:].rearrange("(p c j) f -> c p j f", p=128, j=4)
            ownv = xp_tab[:].rearrange("(p t) f -> t p f", p=128)

            # ---- consts ----
            wt_sb = constp.tile([128, 2, 264], dt.bfloat16)
            for k in range(2):
                nc.sync.dma_start(out=wt_sb[:, k, :], in_=wt_in[k])
            idx_sb = constp.tile([128, TOT // 16], dt.int16)
            nc.sync.dma_start(out=idx_sb[:], in_=idx_in[:])
            bnsc_sb = constp.tile([128, 2], dt.float32)
            nc.sync.dma_start(out=bnsc_sb[:], in_=bnsc_in[:])
            bnsh_sb = constp.tile([128, 2], dt.float32)
            nc.sync.dma_start(out=bnsh_sb[:], in_=bnsh_in[:])
            linw_sb = constp.tile([128, 2, 64], dt.bfloat16)
            for k in range(2):
                nc.sync.dma_start(out=linw_sb[:, k, :], in_=linw_in[k])
            linb_sb = constp.tile([128, 64], dt.float32)
            nc.sync.dma_start(out=linb_sb[:], in_=linb_in[:])
            identf_sb = constp.tile([128, 128], dt.float32)
            nc.sync.dma_start(out=identf_sb[:], in_=identf_in[:])
            att_sb = constp.tile([128, NT, 8], dt.bfloat16)

            # ---- phase A: replicated projection, 4-tile chunks ----
            with (
                tc.tile_pool(name="proj_sb", bufs=10) as psb,
                tc.tile_pool(name="proj_out", bufs=10) as pxp,
                tc.tile_pool(name="proj_ps", bufs=8, space="PSUM") as pps,
            ):
                for c in range(NTC):
                    xt = psb.tile([128, 4, 2, 128], dt.bfloat16)
                    nc.sync.dma_start(out=xt[:], in_=xT4_in[c])
                    xp4 = pxp.tile([128, 4, 256], dt.bfloat16)
                    for j in range(4):
                        ps = pps.tile([128, 264], dt.float32, space="PSUM")
                        nc.tensor.matmul(out=ps[:], lhsT=xt[:, j, 0, :],
                                         rhs=wt_sb[:, 0, :],
                                         start=True, stop=False)
                        nc.tensor.matmul(out=ps[:], lhsT=xt[:, j, 1, :],
                                         rhs=wt_sb[:, 1, :],
                                         start=False, stop=True)
                        if j % 2 == 0:
                            nc.scalar.activation(
                                xp4[:, j, :], ps[:, 0:256],
                                mybir.ActivationFunctionType.Copy)
                            nc.vector.tensor_copy(
                                out=att_sb[:, c * 4 + j, :],
                                in_=ps[:, 256:264])
                        else:
                            nc.vector.tensor_copy(out=xp4[:, j, :],
                                                  in_=ps[:, 0:256])
                            nc.scalar.activation(
                                att_sb[:, c * 4 + j, :], ps[:, 256:264],
                                mybir.ActivationFunctionType.Copy)
                    nc.sync.dma_start(out=tabw[c], in_=xp4[:])

            # ---- phase B: per-block pipeline ----
            with (
                tc.tile_pool(name="gsb", bufs=8) as gsb,
                tc.tile_pool(name="ohsb", bufs=10) as ohsb,
                tc.tile_pool(name="msb", bufs=6) as msb,
                tc.tile_pool(name="osb", bufs=3) as osb,
                tc.tile_pool(name="fsb", bufs=2) as fsb,
                tc.tile_pool(name="aggps", bufs=3, space="PSUM") as aggps,
                tc.tile_pool(name="adstps", bufs=2, space="PSUM") as adstps,
                tc.tile_pool(name="tps", bufs=2, space="PSUM") as tps,
                tc.tile_pool(name="finps", bufs=1, space="PSUM") as finps,
            ):
                qctr = [0]
                gather_insts = []

                def gq():
                    i = qctr[0]
                    qctr[0] += 1
                    return queue_map[i] if queue_map is not None else 0

                for b in range(NB):
                    own_x = osb.tile([128, 256], dt.bfloat16, tag="ox")
                    nc.sync.dma_start(out=own_x[:], in_=ownv[b])

                    agg = aggps.tile([128, 260], dt.float32, space="PSUM")
                    for cls in range(2):
                        ci = b * 2 + cls
                        S = subt_list[ci]
                        NI = ni_list[ci]
                        oE = offs[ci]
                        oW = oE // 16
                        nie = nie_list[ci]
                        xg = gsb.tile([128, SMAX, 256], dt.bfloat16,
                                      tag=f"xg{cls}")
                        src_ap = sup[:, 0:256] if cls == 0 else sup[:, 256:512]
                        for g0 in range(0, S, 8):
                            nrem = min(nie - g0 * 128, 1024)
                            if nrem <= 0:
                                break
                            gs = (nrem + 127) // 128
                            gi = nc.gpsimd.dma_gather(
                                out_ap=xg[:, g0:g0 + gs, :], in_ap=src_ap,
                                idxs_ap=idx_sb[:, oW + g0 * 8:
                                               oW + g0 * 8 + (nrem + 15) // 16],
                                num_idxs=nrem, num_idxs_reg=nrem,
                                elem_size=256, elem_step=512, queue_num=gq())
                            gather_insts.append(gi)
                        # host-precomputed one-hots
                        oh = ohsb.tile([128, SMAX, 128], dt.bfloat16, tag="oh")
                        nc.scalar.dma_start(
                            out=oh[:, 0:S, :],
                            in_=oh_in[:, oE:oE + NI].rearrange(
                                "p (t d) -> p t d", d=128))
                        ohT = ohsb.tile([128, SMAX, 128], dt.bfloat16, tag="ohT")
                        nc.scalar.dma_start(
                            out=ohT[:, 0:S, :],
                            in_=ohT_in[:, oE:oE + NI].rearrange(
                                "p (t e) -> p t e", e=128))
                        # a_src: head-wise row sums via 2x-mode tree adds
                        xg4 = xg[:, 0:S, :].rearrange(
                            "p t (c h) -> p t c h", h=H)
                        tr1 = msb.tile([128, SMAX, 32, 4], dt.bfloat16,
                                       tag="tr1")
                        nc.vector.tensor_tensor(
                            out=tr1[:, 0:S, :, :], in0=xg4[:, :, 0:32, :],
                            in1=xg4[:, :, 32:64, :], op=mybir.AluOpType.add)
                        tr2 = msb.tile([128, SMAX, 16, 4], dt.bfloat16,
                                       tag="tr2")
                        nc.vector.tensor_tensor(
                            out=tr2[:, 0:S, :, :], in0=tr1[:, 0:S, 0:16, :],
                            in1=tr1[:, 0:S, 16:32, :], op=mybir.AluOpType.add)
                        tr3 = msb.tile([128, SMAX, 8, 4], dt.bfloat16,
                                       tag="tr3")
                        nc.vector.tensor_tensor(
                            out=tr3[:, 0:S, :, :], in0=tr2[:, 0:S, 0:8, :],
                            in1=tr2[:, 0:S, 8:16, :], op=mybir.AluOpType.add)
                        asrc = msb.tile([128, SMAX, 4], dt.float32, tag="asrc")
                        nc.vector.reduce_sum(
                            out=asrc[:, 0:S, :],
                            in_=tr3[:, 0:S, :, :].rearrange(
                                "p t c h -> p t h c"),
                            axis=mybir.AxisListType.X)
                        # a_dst per edge via one-hot-transpose matmuls
                        adps = adstps.tile([128, SMAX, 4], dt.float32,
                                           space="PSUM")
                        for t in range(S):
                            nc.tensor.matmul(out=adps[:, t, :],
                                             lhsT=ohT[:, t, :],
                                             rhs=att_sb[:, b, 4:8],
                                             start=True, stop=True)
                        # w = exp(leaky(a_src + a_dst)) -> msg cols 256:260
                        ev = msb.tile([128, SMAX, 4], dt.float32, tag="ev")
                        nc.vector.tensor_tensor(out=ev[:, 0:S, :],
                                                in0=asrc[:, 0:S, :],
                                                in1=adps[:, 0:S, :],
                                                op=mybir.AluOpType.add)
                        lv = msb.tile([128, SMAX, 4], dt.float32, tag="lv")
                        nc.vector.scalar_tensor_tensor(
                            out=lv[:, 0:S, :], in0=ev[:, 0:S, :],
                            scalar=NEG_SLOPE, in1=ev[:, 0:S, :],
                            op0=mybir.AluOpType.mult,
                            op1=mybir.AluOpType.max)
                        msg = msb.tile([128, SMAX, 260], dt.bfloat16,
                                       tag="msg")
                        nc.scalar.activation(msg[:, 0:S, 256:260],
                                             lv[:, 0:S, :],
                                             mybir.ActivationFunctionType.Exp)
                        nc.vector.tensor_tensor(
                            out=msg[:, 0:S, 0:256].rearrange(
                                "p t (c h) -> p t c h", h=H),
                            in0=xg4[:],
                            in1=msg[:, 0:S, 256:260][:, :, None, :]
                                .to_broadcast([128, S, C, H]),
                            op=mybir.AluOpType.mult)
                        for t in range(S):
                            nc.tensor.matmul(
                                out=agg[:], lhsT=oh[:, t, :],
                                rhs=msg[:, t, :],
                                start=(cls == 0 and t == 0),
                                stop=(cls == 1 and t == S - 1))
                    # ---- finalize (self loop + normalize + BN + linear) ----
                    evs = fsb.tile([128, 4], dt.float32, tag="evs")
                    nc.vector.tensor_tensor(out=evs[:], in0=att_sb[:, b, 0:4],
                                            in1=att_sb[:, b, 4:8],
                                            op=mybir.AluOpType.add)
                    lvs = fsb.tile([128, 4], dt.float32, tag="lvs")
                    nc.vector.scalar_tensor_tensor(
                        out=lvs[:], in0=evs[:], scalar=NEG_SLOPE, in1=evs[:],
                        op0=mybir.AluOpType.mult, op1=mybir.AluOpType.max)
                    selfmsg = fsb.tile([128, 260], dt.float32, tag="sm")
                    nc.scalar.activation(selfmsg[:, 256:260], lvs[:],
                                         mybir.ActivationFunctionType.Exp)
                    nc.vector.tensor_tensor(
                        out=selfmsg[:, 0:256].rearrange(
                            "p (c h) -> p c h", h=H),
                        in0=own_x[:].rearrange("p (c h) -> p c h", h=H),
                        in1=selfmsg[:, 256:260][:, None, :].to_broadcast(
                            [128, C, H]),
                        op=mybir.AluOpType.mult)
                    tot = fsb.tile([128, 260], dt.float32, tag="tot")
                    nc.vector.tensor_tensor(out=tot[:], in0=agg[:],
                                            in1=selfmsg[:],
                                            op=mybir.AluOpType.add)
                    rec = fsb.tile([128, 4], dt.float32, tag="rec")
                    nc.vector.reciprocal(rec[:], tot[:, 256:260])
                    gat = fsb.tile([128, 256], dt.float32, tag="gat")
                    nc.vector.tensor_tensor(
                        out=gat[:].rearrange("p (c h) -> p c h", h=H),
                        in0=tot[:, 0:256].rearrange("p (c h) -> p c h", h=H),
                        in1=rec[:, None, :].to_broadcast([128, C, H]),
                        op=mybir.AluOpType.mult)
                    fps = finps.tile([128, 64], dt.float32, space="PSUM")
                    gt = fsb.tile([128, 2, 128], dt.bfloat16, tag="gt")
                    for k in range(2):
                        pst = tps.tile([128, 128], dt.float32, space="PSUM",
                                       tag="pst")
                        nc.tensor.transpose(out=pst[:],
                                            in_=gat[:, k * 128:(k + 1) * 128],
                                            identity=identf_sb[:])
                        nc.scalar.activation(gt[:, k, :], pst[:],
                                             mybir.ActivationFunctionType.Relu,
                                             bias=bnsh_sb[:, k:k + 1],
                                             scale=bnsc_sb[:, k:k + 1])
                        nc.tensor.matmul(out=fps[:], lhsT=gt[:, k, :],
                                         rhs=linw_sb[:, k, :],
                                         start=(k == 0), stop=(k == 1))
                    ob = fsb.tile([128, 64], dt.float32, tag="ob")
                    nc.vector.tensor_tensor(out=ob[:], in0=fps[:],
                                            in1=linb_sb[:],
                                            op=mybir.AluOpType.add)
                    nc.sync.dma_start(
                        out=out_dram[b * 128:(b + 1) * 128, :], in_=ob[:])
    nc.compile()
    return nc, gather_insts


def _queue_map_from_lanes(gather_insts):
    """Pass-1 lane readback: queue k must equal (DMASW lane) % 4."""
    from concourse.tile_scheduler import PROC_NAMES
    qmap = []
    for gi in gather_insts:
        name = PROC_NAMES[gi.ins.bass_scheduled_proc]
        assert name.startswith("DMASW"), name
        qmap.append(int(name[5:]) % 4)
    return qmap


def _install_ntff_shim():
    """Install the axon NTFF profiling hook (missing antenv.axon_hooks shim)."""
    import sys, types
    if "antenv.axon_hooks" in sys.modules:
        return
    m = types.ModuleType("antenv.axon_hooks")
    _h = [None]
    m.set_axon_ntff_profile_hook = lambda h: _h.__setitem__(0, h)
    m.get_axon_ntff_profile_hook = lambda: _h[0]
    sys.modules["antenv.axon_hooks"] = m
    import antenv
    antenv.axon_hooks = m
    from trn_agent_boot.trn_boot import _ntff_profile_via_ctypes
    hook = _ntff_profile_via_ctypes("/opt/axon/libaxon_pjrt.so")
    if hook is not None:
        m.set_axon_ntff_profile_hook(hook)


def kernel(**inputs):
    global LAST_EXEC_NS, LAST_RESULTS
    import os
    from concourse import bass_utils

    trace = os.environ.get("KERNEL_TRACE") == "1"
    if trace:
        try:
            _install_ntff_shim()
            bass_utils.upload_artifacts = lambda tmpdir: "(upload skipped)"
        except Exception as e:
            print("ntff shim failed:", e)
            trace = False

    idx_all, oh_all, ohT_all, subt_cfg = _prep_edges(
        np.asarray(inputs["edge_index"]))
    params = _prep_params(
        inputs["x"], inputs["W"], inputs["att_src"], inputs["att_dst"],
        inputs["gat_bias"], inputs["bn_gamma"], inputs["bn_beta"],
        inputs["bn_mean"], inputs["bn_var"], inputs["lin_W"], inputs["lin_b"])

    nc1, ginsts = _build(subt_cfg)
    nc, _ = _build(subt_cfg, queue_map=_queue_map_from_lanes(ginsts))

    xT_t = params["xT_t"]                    # [NT, 128, 2, 128]
    shared = dict(
        wt_ext=params["wt_ext"], bnsc=params["bnsc"], bnsh=params["bnsh"],
        linw=params["linw"], linb=params["linb"],
        ident_f32=params["ident_f32"])
    in_maps = []
    for p in range(NCORES):
        m = dict(shared)
        rot = np.roll(np.arange(NT), -p * NB)     # tile t holds local rows
        xr = xT_t[rot]
        m["xT4"] = np.ascontiguousarray(
            xr.reshape(NTC, 4, 128, 2, 128).transpose(0, 2, 1, 3, 4))
        m["idx"] = np.ascontiguousarray(idx_all[p])
        m["oh"] = np.ascontiguousarray(oh_all[p])
        m["ohT"] = np.ascontiguousarray(ohT_all[p])
        in_maps.append(m)

    run_kwargs = {}
    if trace:
        run_kwargs = dict(trace=True, tmpdir=os.environ.get(
            "KERNEL_TRACE_DIR", "/tmp/gat_prof"))
        os.makedirs(run_kwargs["tmpdir"], exist_ok=True)
    res = bass_utils.run_bass_kernel_spmd(
        nc, in_maps, core_ids=list(range(NCORES)), **run_kwargs)
    LAST_EXEC_NS = res.exec_time_ns
    LAST_RESULTS = res

    full = np.empty((NPAD, 64), dtype=np.float32)
    for p in range(NCORES):
        full[p * OWN:(p + 1) * OWN] = res.results[p]["out"]
    return full[:N]


# revision 25
# speedup vs baseline: 1.9126x; 1.0193x over previous
"""GAT layer (gnn_message_passing) on 8 Trainium2 NeuronCores — V5.

Strategy (dst-partitioned, replicated projection into rotated local tables):
  * Core p owns dst nodes [p*6272, (p+1)*6272) = 49 blocks of 128.
  * Every core computes the full projected table xp = x @ W.T (bf16,
    feature-permuted j = c*4+h, pre-scaled by att_src) plus an 8-col
    attention sidecar (a_src/a_dst per node — att vectors folded into the
    projection weights, so they come straight out of the matmul). The table
    is stored ROTATED per core: local row r holds global node
    (p*6272 + r) mod 50176, so each core's own dst rows are local rows
    0..6271 with core-independent addressing. PSUM->bf16 copies alternate
    between the vector and scalar engines (both idle during phase A).
  * Edges (no self loops) are bucketed per (dst-block, src-parity) and
    gathered per cell via gpsimd dma_gather (512B rows, superrow int16
    indices), rotated across the 4 SWDGE queues so descriptor generation
    runs on all four Q7 cpu pairs concurrently (~4x). Index padding is -1:
    the Q7 ucode trims trailing negatives, so each core gathers only its
    true edge count (cell sizes are padded to the max over cores).
  * One-hot matrices (dst scatter + transpose) are precomputed on the host
    and DMA-streamed per cell, keeping the vector engine free.
  * Per cell: agg += onehot.T @ [w*xp[src] | w] accumulates messages and the
    softmax denominator per block in PSUM; w = exp(leaky(a_src + a_dst)),
    a_src from head-wise row sums of the pre-scaled gathered rows (2x-mode
    tree adds), a_dst via onehot-transpose matmul against the block sidecar.
  * Self loops: the block's own rows + sidecar give msg_self, added at
    finalize. Finalize: normalize, transpose, fused BN+bias (att_src
    unscale folded into BN scale) + ReLU, final linear -> [6272, 64].
"""

import numpy as np
import ml_dtypes

BF16 = ml_dtypes.bfloat16

N, E, F, H, C = 50000, 800000, 256, 4, 64
NEG_SLOPE = 0.2
BN_EPS = 1e-5
NCORES = 8
BLK = 128
NB = 49
OWN = NB * BLK           # 6272
NPAD = NCORES * OWN      # 50176
NT = NPAD // 128         # 392
NTC = NT // 4            # 98 chunks of 4 tiles

# feature permutation: new index j = c*4 + h  <->  old index h*64 + c
_OLD_OF_NEW = (np.arange(F) % H) * C + (np.arange(F) // H)

LAST_EXEC_NS = None
LAST_RESULTS = None


def _prep_edges(edge_index):
    src = np.asarray(edge_index[0], dtype=np.int64)
    dst = np.asarray(edge_index[1], dtype=np.int64)

    core = dst // OWN
    dst_local = dst - core * OWN
    block = dst_local // BLK
    dst_slot = (dst_local % BLK).astype(np.float32)    # identity slot map
    # partition-major table position on the owning core (rotated tiles):
    # node at (q = src%128, tile_rot) -> pm row q*NT + tile_rot
    q = src % 128
    tile_rot = (src // 128 - core * NB) % NT
    cls = (tile_rot % 2).astype(np.int64)
    pm = q * NT + tile_rot
    gidx = (pm // 2).astype(np.int64)                  # table superrow

    ncell = NB * 2
    cell = core * ncell + block * 2 + cls
    counts = np.bincount(cell, minlength=NCORES * ncell).reshape(NCORES, ncell)
    nie_list = [int(np.ceil(counts[:, ci].max() / 16)) * 16
                for ci in range(ncell)]
    mn_list = [int(counts[:, ci].min()) for ci in range(ncell)]
    subt_list = [(n + 127) // 128 for n in nie_list]
    ni_list = [s * 128 for s in subt_list]
    nie_list = ni_list
    offs = np.zeros(ncell + 1, dtype=np.int64)
    np.cumsum(ni_list, out=offs[1:])
    TOT = int(offs[-1])

    order = np.argsort(cell, kind="stable")
    sorted_cell = cell[order]
    cell_starts = np.zeros(NCORES * ncell + 1, dtype=np.int64)
    np.cumsum(counts.reshape(-1), out=cell_starts[1:])
    rank = np.arange(len(order)) - cell_starts[sorted_cell]
    ci_of = sorted_cell % ncell
    core_of = sorted_cell // ncell
    flat_pos = offs[ci_of] + rank                      # position within core

    gidx_pad = np.zeros((NCORES, TOT), dtype=np.int64)
    gidx_pad[core_of, flat_pos] = gidx[order]
    dstm_pad = np.full((NCORES, TOT), 200.0, dtype=np.float32)
    dstm_pad[core_of, flat_pos] = dst_slot[order]

    # wrapped gather indices [16, TOT//16] -> replicated x8 across partitions
    g = gidx_pad.astype(np.int16).reshape(NCORES, TOT // 16, 16)
    g = np.ascontiguousarray(g.transpose(0, 2, 1))
    idx_all = np.tile(g, (1, 8, 1))                    # [8, 128, TOT//16]

    # host-built one-hots, bf16:
    #   oh[p, (t,d)]  = (dstm_gather[p, t] == d)   (partition = edge lane)
    #   ohT[d, (t,e)] = (dstm_flat[t*128+e] == d)  (partition = dst slot)
    dst_w = np.empty((NCORES, 128, TOT // 128), dtype=np.float32)
    for ci in range(ncell):
        seg = dstm_pad[:, offs[ci]:offs[ci + 1]].reshape(
            NCORES, subt_list[ci], 128)
        dst_w[:, :, offs[ci] // 128:offs[ci + 1] // 128] = \
            seg.transpose(0, 2, 1)
    dvals = np.arange(128, dtype=np.float32)
    oh_all = np.empty((NCORES, 128, TOT), dtype=BF16)
    ohT_all = np.empty((NCORES, 128, TOT), dtype=BF16)
    for p in range(NCORES):
        oh = (dst_w[p][:, :, None] == dvals).astype(BF16)   # [128, T/128, 128]
        oh_all[p] = oh.reshape(128, TOT)
        ohT_all[p] = (dvals[:, None] == dstm_pad[p][None, :]).astype(BF16)

    return idx_all, oh_all, ohT_all, (subt_list, nie_list, mn_list)


def _prep_params(x, W, att_src, att_dst, gat_bias, bn_gamma, bn_beta,
                 bn_mean, bn_var, lin_W, lin_b):
    f32 = np.float32
    W = np.asarray(W, f32)
    att_src_f = np.asarray(att_src, f32).reshape(H * C)      # index h*64+c
    att_src_hc = np.asarray(att_src, f32)                    # [H, C]
    att_dst_hc = np.asarray(att_dst, f32)

    wt = W.T                                                 # [in, out_old]
    wt_perm = wt[:, _OLD_OF_NEW] * att_src_f[_OLD_OF_NEW][None, :]
    aw_src = np.zeros((F, H), dtype=f32)
    aw_dst = np.zeros((F, H), dtype=f32)
    for h in range(H):
        aw_src[:, h] = W[h * C:(h + 1) * C, :].T @ att_src_hc[h]
        aw_dst[:, h] = W[h * C:(h + 1) * C, :].T @ att_dst_hc[h]
    wt_full = np.concatenate([wt_perm, aw_src, aw_dst], axis=1)  # [256, 264]
    wt_ext = np.ascontiguousarray(wt_full.reshape(2, 128, 264)).astype(BF16)

    xT = np.zeros((F, NPAD), dtype=f32)
    xT[:, :N] = np.asarray(x, f32).T
    xT_t = np.ascontiguousarray(
        xT.reshape(2, 128, NT, 128).transpose(2, 1, 0, 3)).astype(BF16)

    bnscale = np.asarray(bn_gamma, f32) / np.sqrt(np.asarray(bn_var, f32) + BN_EPS)
    bnshift = ((np.asarray(gat_bias, f32) - np.asarray(bn_mean, f32)) * bnscale
               + np.asarray(bn_beta, f32))
    bnsc_f = bnscale[_OLD_OF_NEW] / att_src_f[_OLD_OF_NEW]   # fold unscale
    bnsc = np.ascontiguousarray(bnsc_f.reshape(2, 128).T)
    bnsh = np.ascontiguousarray(bnshift[_OLD_OF_NEW].reshape(2, 128).T)

    linw = np.asarray(lin_W, f32).T[_OLD_OF_NEW, :]
    linw_t = np.ascontiguousarray(linw.reshape(2, 128, 64)).astype(BF16)
    linb_rep = np.tile(np.asarray(lin_b, f32)[None, :], (128, 1))

    ident_f32 = np.eye(128, dtype=np.float32)

    return dict(xT_t=xT_t, wt_ext=wt_ext, bnsc=bnsc.astype(f32),
                bnsh=bnsh.astype(f32), linw=linw_t, linb=linb_rep.astype(f32),
                ident_f32=ident_f32)


def _build(subt_cfg, queue_map=None):
    import concourse.bacc as bacc
    import concourse.mybir as mybir
    import concourse.tile as tile

    dt = mybir.dt
    subt_list, nie_list, mn_list = subt_cfg
    ni_list = [s * 128 for s in subt_list]
    offs = [0]
    for n in ni_list:
        offs.append(offs[-1] + n)
    TOT = offs[-1]
    SMAX = max(subt_list)

    nc = bacc.Bacc("TRN2", target_bir_lowering=False, debug=False,
                   enable_asserts=False, num_devices=NCORES,
                   num_swdge_queues=4)

    xT4_in = nc.dram_tensor("xT4", [NTC, 128, 4, 2, 128], dt.bfloat16,
                            kind="ExternalInput")
    wt_in = nc.dram_tensor("wt_ext", [2, 128, 264], dt.bfloat16,
                           kind="ExternalInput")
    bnsc_in = nc.dram_tensor("bnsc", [128, 2], dt.float32, kind="ExternalInput")
    bnsh_in = nc.dram_tensor("bnsh", [128, 2], dt.float32, kind="ExternalInput")
    linw_in = nc.dram_tensor("linw", [2, 128, 64], dt.bfloat16, kind="ExternalInput")
    linb_in = nc.dram_tensor("linb", [128, 64], dt.float32, kind="ExternalInput")
    identf_in = nc.dram_tensor("ident_f32", [128, 128], dt.float32, kind="ExternalInput")
    idx_in = nc.dram_tensor("idx", [128, TOT // 16], dt.int16, kind="ExternalInput")
    oh_in = nc.dram_tensor("oh", [128, TOT], dt.bfloat16, kind="ExternalInput")
    ohT_in = nc.dram_tensor("ohT", [128, TOT], dt.bfloat16, kind="ExternalInput")
    out_dram = nc.dram_tensor("out", [OWN, 64], dt.float32, kind="ExternalOutput")

    with tile.TileContext(nc) as tc:
        with (
            tc.tile_pool(name="dram", bufs=1, space="DRAM") as dramp,
            tc.tile_pool(name="const", bufs=1) as constp,
        ):
            xp_tab = dramp.tile([NPAD, 256], dt.bfloat16)
            sup = xp_tab[:].rearrange("(s two) f -> s (two f)", two=2)
            # PM row p*NT + t: write chunks contiguously per partition,
            # read own-block rows as [t, p, f]
            tabw = xp_tab[:].rearrange("(p c j) f -> c p j f", p=128, j=4)
            ownv = xp_tab[:].rearrange("(p t) f -> t p f", p=128)

            # ---- consts ----
            wt_sb = constp.tile([128, 2, 264], dt.bfloat16)
            for k in range(2):
                nc.sync.dma_start(out=wt_sb[:, k, :], in_=wt_in[k])
            idx_sb = constp.tile([128, TOT // 16], dt.int16)
            nc.sync.dma_start(out=idx_sb[:], in_=idx_in[:])
            bnsc_sb = constp.tile([128, 2], dt.float32)
            nc.sync.dma_start(out=bnsc_sb[:], in_=bnsc_in[:])
            bnsh_sb = constp.tile([128, 2], dt.float32)
            nc.sync.dma_start(out=bnsh_sb[:], in_=bnsh_in[:])
            linw_sb = constp.tile([128, 2, 64], dt.bfloat16)
            for k in range(2):
                nc.sync.dma_start(out=linw_sb[:, k, :], in_=linw_in[k])
            linb_sb = constp.tile([128, 64], dt.float32)
            nc.sync.dma_start(out=linb_sb[:], in_=linb_in[:])
            identf_sb = constp.tile([128, 128], dt.float32)
            nc.sync.dma_start(out=identf_sb[:], in_=identf_in[:])
            att_sb = constp.tile([128, NT, 8], dt.bfloat16)

            # ---- phase A: replicated projection, 4-tile chunks ----
            with (
                tc.tile_pool(name="proj_sb", bufs=10) as psb,
                tc.tile_pool(name="proj_out", bufs=10) as pxp,
                tc.tile_pool(name="proj_ps", bufs=8, space="PSUM") as pps,
            ):
                for c in range(NTC):
                    xt = psb.tile([128, 4, 2, 128], dt.bfloat16)
                    nc.sync.dma_start(out=xt[:], in_=xT4_in[c])
                    xp4 = pxp.tile([128, 4, 256], dt.bfloat16)
                    for j in range(4):
                        ps = pps.tile([128, 264], dt.float32, space="PSUM")
                        nc.tensor.matmul(out=ps[:], lhsT=xt[:, j, 0, :],
                                         rhs=wt_sb[:, 0, :],
                                         start=True, stop=False)
                        nc.tensor.matmul(out=ps[:], lhsT=xt[:, j, 1, :],
                                         rhs=wt_sb[:, 1, :],
                                         start=False, stop=True)
                        if j % 2 == 0:
                            nc.scalar.activation(
                                xp4[:, j, :], ps[:, 0:256],
                                mybir.ActivationFunctionType.Copy)
                            nc.vector.tensor_copy(
                                out=att_sb[:, c * 4 + j, :],
                                in_=ps[:, 256:264])
                        else:
                            nc.vector.tensor_copy(out=xp4[:, j, :],
                                                  in_=ps[:, 0:256])
                            nc.scalar.activation(
                                att_sb[:, c * 4 + j, :], ps[:, 256:264],
                                mybir.ActivationFunctionType.Copy)
                    nc.sync.dma_start(out=tabw[c], in_=xp4[:])

            # ---- phase B: per-block pipeline ----
            with (
                tc.tile_pool(name="gsb", bufs=8) as gsb,
                tc.tile_pool(name="ohsb", bufs=10) as ohsb,
                tc.tile_pool(name="msb", bufs=6) as msb,
                tc.tile_pool(name="osb", bufs=3) as osb,
                tc.tile_pool(name="fsb", bufs=2) as fsb,
                tc.tile_pool(name="aggps", bufs=3, space="PSUM") as aggps,
                tc.tile_pool(name="adstps", bufs=2, space="PSUM") as adstps,
                tc.tile_pool(name="tps", bufs=2, space="PSUM") as tps,
                tc.tile_pool(name="finps", bufs=1, space="PSUM") as finps,
            ):
                qctr = [0]
                gather_insts = []

                def gq():
                    i = qctr[0]
                    qctr[0] += 1
                    return queue_map[i] if queue_map is not None else 0

                for b in range(NB):
                    own_x = osb.tile([128, 256], dt.bfloat16, tag="ox")
                    nc.sync.dma_start(out=own_x[:], in_=ownv[b])

                    agg = aggps.tile([128, 260], dt.float32, space="PSUM")
                    for cls in range(2):
                        ci = b * 2 + cls
                        S = subt_list[ci]
                        NI = ni_list[ci]
                        oE = offs[ci]
                        oW = oE // 16
                        nie = nie_list[ci]
                        xg = gsb.tile([128, SMAX, 256], dt.bfloat16,
                                      tag=f"xg{cls}")
                        src_ap = sup[:, 0:256] if cls == 0 else sup[:, 256:512]
                        for g0 in range(0, S, 8):
                            nrem = min(nie - g0 * 128, 1024)
                            if nrem <= 0:
                                break
                            gs = (nrem + 127) // 128
                            gi = nc.gpsimd.dma_gather(
                                out_ap=xg[:, g0:g0 + gs, :], in_ap=src_ap,
                                idxs_ap=idx_sb[:, oW + g0 * 8:
                                               oW + g0 * 8 + (nrem + 15) // 16],
                                num_idxs=nrem, num_idxs_reg=nrem,
                                elem_size=256, elem_step=512, queue_num=gq())
                            gather_insts.append(gi)
                        # host-precomputed one-hots
                        oh = ohsb.tile([128, SMAX, 128], dt.bfloat16, tag="oh")
                        nc.scalar.dma_start(
                            out=oh[:, 0:S, :],
                            in_=oh_in[:, oE:oE + NI].rearrange(
                                "p (t d) -> p t d", d=128))
                        ohT = ohsb.tile([128, SMAX, 128], dt.bfloat16, tag="ohT")
                        nc.scalar.dma_start(
                            out=ohT[:, 0:S, :],
                            in_=ohT_in[:, oE:oE + NI].rearrange(
                                "p (t e) -> p t e", e=128))
                        # a_src: head-wise row sums via 2x-mode tree adds
                        xg4 = xg[:, 0:S, :].rearrange(
                            "p t (c h) -> p t c h", h=H)
                        tr1 = msb.tile([128, SMAX, 32, 4], dt.bfloat16,
                                       tag="tr1")
                        nc.vector.tensor_tensor(
                            out=tr1[:, 0:S, :, :], in0=xg4[:, :, 0:32, :],
                            in1=xg4[:, :, 32:64, :], op=mybir.AluOpType.add)
                        tr2 = msb.tile([128, SMAX, 16, 4], dt.bfloat16,
                                       tag="tr2")
                        nc.vector.tensor_tensor(
                            out=tr2[:, 0:S, :, :], in0=tr1[:, 0:S, 0:16, :],
                            in1=tr1[:, 0:S, 16:32, :], op=mybir.AluOpType.add)
                        tr3 = msb.tile([128, SMAX, 8, 4], dt.bfloat16,
                                       tag="tr3")
                        nc.vector.tensor_tensor(
                            out=tr3[:, 0:S, :, :], in0=tr2[:, 0:S, 0:8, :],
                            in1=tr2[:, 0:S, 8:16, :], op=mybir.AluOpType.add)
                        asrc = msb.tile([128, SMAX, 4], dt.float32, tag="asrc")
                        nc.vector.reduce_sum(
                            out=asrc[:, 0:S, :],
                            in_=tr3[:, 0:S, :, :].rearrange(
                                "p t c h -> p t h c"),
                            axis=mybir.AxisListType.X)
                        # a_dst per edge via one-hot-transpose matmuls
                        adps = adstps.tile([128, SMAX, 4], dt.float32,
                                           space="PSUM")
                        for t in range(S):
                            nc.tensor.matmul(out=adps[:, t, :],
                                             lhsT=ohT[:, t, :],
                                             rhs=att_sb[:, b, 4:8],
                                             start=True, stop=True)
                        # w = exp(leaky(a_src + a_dst)) -> msg cols 256:260
                        ev = msb.tile([128, SMAX, 4], dt.float32, tag="ev")
                        nc.vector.tensor_tensor(out=ev[:, 0:S, :],
                                                in0=asrc[:, 0:S, :],
                                                in1=adps[:, 0:S, :],
                                                op=mybir.AluOpType.add)
                        lv = msb.tile([128, SMAX, 4], dt.float32, tag="lv")
                        nc.vector.scalar_tensor_tensor(
                            out=lv[:, 0:S, :], in0=ev[:, 0:S, :],
                            scalar=NEG_SLOPE, in1=ev[:, 0:S, :],
                            op0=mybir.AluOpType.mult,
                            op1=mybir.AluOpType.max)
                        msg = msb.tile([128, SMAX, 260], dt.bfloat16,
                                       tag="msg")
                        nc.scalar.activation(msg[:, 0:S, 256:260],
                                             lv[:, 0:S, :],
                                             mybir.ActivationFunctionType.Exp)
                        nc.vector.tensor_tensor(
                            out=msg[:, 0:S, 0:256].rearrange(
                                "p t (c h) -> p t c h", h=H),
                            in0=xg4[:],
                            in1=msg[:, 0:S, 256:260][:, :, None, :]
                                .to_broadcast([128, S, C, H]),
                            op=mybir.AluOpType.mult)
                        for t in range(S):
                            nc.tensor.matmul(
                                out=agg[:], lhsT=oh[:, t, :],
                                rhs=msg[:, t, :],
                                start=(cls == 0 and t == 0),
                                stop=(cls == 1 and t == S - 1))
                    # ---- finalize (self loop + normalize + BN + linear) ----
                    evs = fsb.tile([128, 4], dt.float32, tag="evs")
                    nc.vector.tensor_tensor(out=evs[:], in0=att_sb[:, b, 0:4],
                                            in1=att_sb[:, b, 4:8],
                                            op=mybir.AluOpType.add)
                    lvs = fsb.tile([128, 4], dt.float32, tag="lvs")
                    nc.vector.scalar_tensor_tensor(
                        out=lvs[:], in0=evs[:], scalar=NEG_SLOPE, in1=evs[:],
                        op0=mybir.AluOpType.mult, op1=mybir.AluOpType.max)
                    selfmsg = fsb.tile([128, 260], dt.float32, tag="sm")
                    nc.scalar.activation(selfmsg[:, 256:260], lvs[:],
                                         mybir.ActivationFunctionType.Exp)
                    nc.vector.tensor_tensor(
                        out=selfmsg[:, 0:256].rearrange(
                            "p (c h) -> p c h", h=H),
                        in0=own_x[:].rearrange("p (c h) -> p c h", h=H),
                        in1=selfmsg[:, 256:260][:, None, :].to_broadcast(
                            [128, C, H]),
                        op=mybir.AluOpType.mult)
                    tot = fsb.tile([128, 260], dt.float32, tag="tot")
                    nc.vector.tensor_tensor(out=tot[:], in0=agg[:],
                                            in1=selfmsg[:],
                                            op=mybir.AluOpType.add)
                    rec = fsb.tile([128, 4], dt.float32, tag="rec")
                    nc.vector.reciprocal(rec[:], tot[:, 256:260])
                    gat = fsb.tile([128, 256], dt.float32, tag="gat")
                    nc.vector.tensor_tensor(
                        out=gat[:].rearrange("p (c h) -> p c h", h=H),
                        in0=tot[:, 0:256].rearrange("p (c h) -> p c h", h=H),
                        in1=rec[:, None, :].to_broadcast([128, C, H]),
                        op=mybir.AluOpType.mult)
                    fps = finps.tile([128, 64], dt.float32, space="PSUM")
                    gt = fsb.tile([128, 2, 128], dt.bfloat16, tag="gt")
                    for k in range(2):
                        pst = tps.tile([128, 128], dt.float32, space="PSUM",
                                       tag="pst")
                        nc.tensor.transpose(out=pst[:],
                                            in_=gat[:, k * 128:(k + 1) * 128],
                                            identity=identf_sb[:])
                        nc.scalar.activation(gt[:, k, :], pst[:],
                                             mybir.ActivationFunctionType.Relu,
                                             bias=bnsh_sb[:, k:k + 1],
                                             scale=bnsc_sb[:, k:k + 1])
                        nc.tensor.matmul(out=fps[:], lhsT=gt[:, k, :],
                                         rhs=linw_sb[:, k, :],
                                         start=(k == 0), stop=(k == 1))
                    ob = fsb.tile([128, 64], dt.float32, tag="ob")
                    nc.vector.tensor_tensor(out=ob[:], in0=fps[:],
                                            in1=linb_sb[:],
                                            op=mybir.AluOpType.add)
                    nc.sync.dma_start(
                        out=out_dram[b * 128:(b + 1) * 128, :], in_=ob[:])
    nc.compile()
    return nc, gather_insts


def _queue_map_from_lanes(gather_insts):
    """Pass-1 lane readback: queue k must equal (DMASW lane) % 4."""
    from concourse.tile_scheduler import PROC_NAMES
    qmap = []
    for gi in gather_insts:
        name = PROC_NAMES[gi.ins.bass_scheduled_proc]
        assert name.startswith("DMASW"), name
        qmap.append(int(name[5:]) % 4)
    return qmap


def _install_ntff_shim():
    """Install the axon NTFF profiling hook (missing antenv.axon_hooks shim)."""
    import sys, types
    if "antenv.axon_hooks" in sys.modules:
        return
    m = types.ModuleType("antenv.axon_hooks")
    _h = [None]
    m.set_axon_ntff_profile_hook = lambda h: _h.__setitem__(0, h)
    m.get_axon_ntff_profile_hook = lambda: _h[0]
    sys.modules["antenv.axon_hooks"] = m
    import antenv
    antenv.axon_hooks = m
    from trn_agent_boot.trn_boot import _ntff_profile_via_ctypes
    hook = _ntff_profile_via_ctypes("/opt/axon/libaxon_pjrt.so")
    if hook is not None:
        m.set_axon_ntff_profile_hook(hook)


def kernel(**inputs):
    global LAST_EXEC_NS, LAST_RESULTS
    import os
    from concourse import bass_utils

    trace = os.environ.get("KERNEL_TRACE") == "1"
    if trace:
        try:
            _install_ntff_shim()
            bass_utils.upload_artifacts = lambda tmpdir: "(upload skipped)"
        except Exception as e:
            print("ntff shim failed:", e)
            trace = False

    idx_all, oh_all, ohT_all, subt_cfg = _prep_edges(
        np.asarray(inputs["edge_index"]))
    params = _prep_params(
        inputs["x"], inputs["W"], inputs["att_src"], inputs["att_dst"],
        inputs["gat_bias"], inputs["bn_gamma"], inputs["bn_beta"],
        inputs["bn_mean"], inputs["bn_var"], inputs["lin_W"], inputs["lin_b"])

    nc1, ginsts = _build(subt_cfg)
    nc, _ = _build(subt_cfg, queue_map=_queue_map_from_lanes(ginsts))

    xT_t = params["xT_t"]                    # [NT, 128, 2, 128]
    shared = dict(
        wt_ext=params["wt_ext"], bnsc=params["bnsc"], bnsh=params["bnsh"],
        linw=params["linw"], linb=params["linb"],
        ident_f32=params["ident_f32"])
    in_maps = []
    for p in range(NCORES):
        m = dict(shared)
        rot = np.roll(np.arange(NT), -p * NB)     # tile t holds local rows
        xr = xT_t[rot]
        m["xT4"] = np.ascontiguousarray(
            xr.reshape(NTC, 4, 128, 2, 128).transpose(0, 2, 1, 3, 4))
        m["idx"] = np.ascontiguousarray(idx_all[p])
        m["oh"] = np.ascontiguousarray(oh_all[p])
        m["ohT"] = np.ascontiguousarray(ohT_all[p])
        in_maps.append(m)

    run_kwargs = {}
    if trace:
        run_kwargs = dict(trace=True, tmpdir=os.environ.get(
            "KERNEL_TRACE_DIR", "/tmp/gat_prof"))
        os.makedirs(run_kwargs["tmpdir"], exist_ok=True)
    res = bass_utils.run_bass_kernel_spmd(
        nc, in_maps, core_ids=list(range(NCORES)), **run_kwargs)
    LAST_EXEC_NS = res.exec_time_ns
    LAST_RESULTS = res

    full = np.empty((NPAD, 64), dtype=np.float32)
    for p in range(NCORES):
        full[p * OWN:(p + 1) * OWN] = res.results[p]["out"]
    return full[:N]
